# revision 1
# baseline (speedup 1.0000x reference)
"""Trainium2 Bass kernel for nn_EntropyLM (wavelet-coeff mixer + chunked MHA + output proj).

Strategy: data-parallel over the 16 independent (batch x chunk) blocks, 2 per
NeuronCore.  All matmuls run in bf16 on the PE with fp32 PSUM accumulation;
layernorm / softmax statistics are computed in fp32.

Layout convention per chunk (CHUNK=1024 tokens, H=1024 features):
  * Linear layers contract over features, so the activation operand of each
    matmul must be feature-major ("T" tensors: [feat_part, token_free]).
  * LN / softmax reductions run along the free axis, so those stages use
    token-major tensors ([token_part, feat_free]).
  * Attention scores are computed directly transposed (ST = K @ Q^T, i.e.
    [k_part, q_free]); exp(ST) is then exactly the lhsT operand that the
    PV matmul needs, which avoids any on-chip transpose of the score matrix.
    The softmax denominator is computed with a ones-vector matmul (partition
    reduction on the PE) and applied per-partition after PV.
  * Orientation changes of bf16 activations go through the DMA xbar
    transpose engine (dma_start_transpose), never through the PE.
"""

import numpy as np
import ml_dtypes

B, S, H, G, W = 4, 4096, 1024, 256, 8
CHUNK = 1024
NUM_HEADS = 4
HD = H // NUM_HEADS          # 256 per-head dim
HM = H // 2                  # 512 mixer hidden
N_CHUNKS = B * (S // CHUNK)  # 16 independent chunks
N_CORES = 8
CPC = N_CHUNKS // N_CORES    # 2 chunks per core
NT = CHUNK // 128            # 8 token tiles
KH = H // 128                # 8 feature tiles (H)
KM = HM // 128               # 4 feature tiles (HM)
EPS = 1e-5
BF16 = ml_dtypes.bfloat16

_COMPILED = None


def _build(debug=False):
    import concourse.bass as bass  # noqa: F401
    import concourse.tile as tile
    from concourse import bacc, mybir

    bf = mybir.dt.bfloat16
    fp16 = mybir.dt.float16
    f32 = mybir.dt.float32
    Alu = mybir.AluOpType
    Act = mybir.ActivationFunctionType

    nc = bacc.Bacc("TRN2", target_bir_lowering=False, debug=False,
                   enable_asserts=True, num_devices=N_CORES)

    # ---- DRAM tensors (per-core views; same NEFF on all 8 cores) ----
    xt = nc.dram_tensor("xt", [CPC, H, CHUNK], bf, kind="ExternalInput")
    kernT = nc.dram_tensor("kernt", [H, W], bf, kind="ExternalInput")
    w1a = nc.dram_tensor("w1a", [W + 1, HM], bf, kind="ExternalInput")
    gln = nc.dram_tensor("gln", [128, KM], f32, kind="ExternalInput")
    bln = nc.dram_tensor("bln", [128, KM], f32, kind="ExternalInput")
    w2 = nc.dram_tensor("w2", [HM, H], bf, kind="ExternalInput")
    b2c = nc.dram_tensor("b2c", [128, KH], f32, kind="ExternalInput")
    wq = nc.dram_tensor("wq", [H, H], bf, kind="ExternalInput")
    wk = nc.dram_tensor("wk", [H, H], bf, kind="ExternalInput")
    wv = nc.dram_tensor("wv", [H, H], bf, kind="ExternalInput")
    wo = nc.dram_tensor("wo", [H, H], bf, kind="ExternalInput")
    gw = nc.dram_tensor("gw", [H, G], bf, kind="ExternalInput")
    bw = nc.dram_tensor("bw", [128, G], f32, kind="ExternalInput")
    y = nc.dram_tensor("y", [CPC, CHUNK, G], f32, kind="ExternalOutput")
    dbg = {}
    if debug:
        for nm, shp, dt in [
            ("dcoef", [W + 1, CHUNK], bf),
            ("dhidT", [128, KM, CHUNK], bf), ("dmixT", [128, KH, CHUNK], bf),
            ("dmixN", [128, NT, H], bf), ("dqT", [128, KH, CHUNK], bf),
            ("dkT", [128, KH, CHUNK], bf), ("dvN", [128, NT, H], fp16),
            ("det", [128, KH, CHUNK], fp16), ("docat", [128, NT, H], bf),
            ("dres", [128, NT, H], bf), ("dz", [128, NT, H], bf),
            ("dzT", [128, KH, CHUNK], bf), ("dsq", [128, NT], f32),
        ]:
            dbg[nm] = nc.dram_tensor(nm, shp, dt, kind="ExternalOutput")

    with tile.TileContext(nc) as tc:
        with (
            tc.tile_pool(name="wp", bufs=1) as wp,
            tc.tile_pool(name="ws", bufs=1) as ws,
            tc.tile_pool(name="sm", bufs=2) as sm,
            tc.tile_pool(name="ps", bufs=3, space="PSUM") as ps,
            tc.tile_pool(name="ps2", bufs=2, space="PSUM") as ps2,
        ):
            # ---------- persistent weights ----------
            kt_sb = wp.tile([128, KH, W], bf, tag="ktw")
            nc.sync.dma_start(kt_sb[:], kernT.ap().rearrange("(i p) w -> p i w", p=128))
            w1a_sb = wp.tile([W + 1, HM], bf, tag="w1a")
            nc.sync.dma_start(w1a_sb[:], w1a.ap())
            gln_sb = wp.tile([128, KM], f32, tag="gln")
            nc.sync.dma_start(gln_sb[:], gln.ap())
            bln_sb = wp.tile([128, KM], f32, tag="bln")
            nc.sync.dma_start(bln_sb[:], bln.ap())
            b2_sb = wp.tile([128, KH], f32, tag="b2")
            nc.sync.dma_start(b2_sb[:], b2c.ap())
            gw_sb = wp.tile([128, KH, G], bf, tag="gw")
            nc.sync.dma_start(gw_sb[:], gw.ap().rearrange("(i p) g -> p i g", p=128))
            bw_sb = wp.tile([128, G], f32, tag="bw")
            nc.sync.dma_start(bw_sb[:], bw.ap())
            ones_sb = wp.tile([128, 1], fp16, tag="ones")
            nc.vector.memset(ones_sb[:], 1.0)
            eps_sb = wp.tile([128, 1], f32, tag="eps")
            nc.vector.memset(eps_sb[:], EPS)

            def stream_w(src):
                dst = ws.tile([128, KH, H], bf, tag="wstream", bufs=2, name="wst")
                nc.sync.dma_start(dst[:], src.ap().rearrange("(i p) m -> p i m", p=128))
                return dst

            # ---------- stage 1 (both chunks up front): wavelet coeffs ----------
            # Running chunk 1's input load + tiny coeff matmuls during chunk 0's
            # mixer window removes the chunk-boundary DMA stall.
            coefs = []
            for c in range(CPC):
                xts = ws.tile([128, KH, CHUNK], bf, tag="xts_et", bufs=2)
                for ii in range(2):
                    nc.sync.dma_start(
                        xts[:, ii * 4:(ii + 1) * 4, :],
                        xt.ap()[c, ii * 512:(ii + 1) * 512, :].rearrange(
                            "(i p) t -> p i t", p=128))
                coef = ws.tile([W + 1, CHUNK], bf, tag="coef", bufs=2)
                # row W is the constant 1.0 bias row for the folded mix_b1
                nc.gpsimd.memset(coef[:, :], 1.0)
                for n in range(2):
                    cps = ps.tile([128, 512], f32, tag="mm")
                    for i in range(KH):
                        nc.tensor.matmul(cps[:W, :], kt_sb[:, i, :],
                                         xts[:, i, n * 512:(n + 1) * 512],
                                         start=(i == 0), stop=(i == KH - 1))
                    nc.scalar.copy(coef[:W, n * 512:(n + 1) * 512], cps[:W, :])
                coefs.append(coef)

            for c in range(CPC):
                coef = coefs[c]
                w2s = ws.tile([128, KM, H], bf, tag="wstream", bufs=2, name="w2s")
                nc.sync.dma_start(w2s[:], w2.ap().rearrange("(i p) m -> p i m", p=128))
                wq_sb = stream_w(wq)
                wk_sb = stream_w(wk)
                if debug and c == 0:
                    nc.sync.dma_start(dbg["dcoef"].ap(), coef[:])
                # ---------- stage 2: mixer hidden + LN + gelu -> hidT ----------
                # z1 = (pre-m)*inv in token-major (stats per-partition), then
                # transpose; gamma/beta + gelu applied feature-major where
                # they are per-partition -> one fused TS + in-place gelu.
                hidT = ws.tile([128, KM, CHUNK], bf, tag="hidT")
                for t in range(NT):
                    hps = ps.tile([128, 512], f32, tag="mm")
                    nc.tensor.matmul(hps[:], coef[:, t * 128:(t + 1) * 128],
                                     w1a_sb[:], start=True, stop=True)
                    st6 = sm.tile([128, 6], f32, tag="st6")
                    nc.vector.bn_stats(st6[:], hps[:])
                    mv = sm.tile([128, 2], f32, tag="mv")
                    nc.vector.bn_aggr(mv[:], st6[:])
                    sq = sm.tile([128, 1], f32, tag="sq")
                    nc.scalar.activation(sq[:], mv[:, 1:2], Act.Sqrt, bias=eps_sb[:])
                    iv = sm.tile([128, 1], f32, tag="iv")
                    nc.vector.reciprocal(iv[:], sq[:])
                    tmp = sm.tile([128, HM], bf, tag="mtmp")
                    nc.vector.tensor_scalar(tmp[:], hps[:],
                                            mv[:, 0:1], iv[:],
                                            op0=Alu.subtract, op1=Alu.mult)
                    nc.sync.dma_start_transpose(hidT[:, :, t * 128:(t + 1) * 128],
                                                tmp[:])
                for nh in range(2):
                    for ki in range(KM):
                        sl = hidT[:, ki, nh * 512:(nh + 1) * 512]
                        nc.vector.tensor_scalar(sl, sl,
                                                gln_sb[:, ki:ki + 1], bln_sb[:, ki:ki + 1],
                                                op0=Alu.mult, op1=Alu.add)
                        nc.scalar.activation(sl, sl, Act.Gelu)

                if debug and c == 0:
                    nc.sync.dma_start(dbg["dhidT"].ap(), hidT[:])
                # ---------- stage 3: mixedT (+b2) and mixed_nat ----------
                mixT = ws.tile([128, KH, CHUNK], bf, tag="mixT_z", bufs=2)
                for n in range(2):
                    for m in range(KH):
                        mps = ps.tile([128, 512], f32, tag="mm")
                        for ki in range(KM):
                            nc.tensor.matmul(mps[:], w2s[:, ki, m * 128:(m + 1) * 128],
                                             hidT[:, ki, n * 512:(n + 1) * 512],
                                             start=(ki == 0), stop=(ki == KM - 1))
                        nc.vector.tensor_scalar(mixT[:, m, n * 512:(n + 1) * 512], mps[:],
                                                b2_sb[:, m:m + 1], None, op0=Alu.add)
                mixN = ws.tile([128, NT, H], bf, tag="mixN")
                for m in range(KH):
                    nc.sync.dma_start_transpose(mixN[:, :, m * 128:(m + 1) * 128],
                                                mixT[:, m, :])

                if debug and c == 0:
                    nc.sync.dma_start(dbg["dmixT"].ap(), mixT[:])
                    nc.sync.dma_start(dbg["dmixN"].ap(), mixN[:])
                # ---------- stage 4: qT, kT, v ----------
                qT = ws.tile([128, KH, CHUNK], bf, tag="qT_otc")
                kT = ws.tile([128, KH, CHUNK], bf, tag="kT_zT")
                for (dst, wsb, on_act) in ((qT, wq_sb, True), (kT, wk_sb, False)):
                    for n in range(2):
                        for m in range(KH):
                            qps = ps.tile([128, 512], f32, tag="mm")
                            for ki in range(KH):
                                nc.tensor.matmul(qps[:], wsb[:, ki, m * 128:(m + 1) * 128],
                                                 mixT[:, ki, n * 512:(n + 1) * 512],
                                                 start=(ki == 0), stop=(ki == KH - 1))
                            if on_act:
                                nc.scalar.copy(dst[:, m, n * 512:(n + 1) * 512], qps[:])
                            else:
                                nc.vector.tensor_copy(dst[:, m, n * 512:(n + 1) * 512], qps[:])
                wv_sb = stream_w(wv)
                vN = ws.tile([128, NT, H], fp16, tag="hp_v")
                for t in range(NT):
                    for n in range(2):
                        vps = ps.tile([128, 512], f32, tag="mm")
                        for ki in range(KH):
                            nc.tensor.matmul(vps[:], mixT[:, ki, t * 128:(t + 1) * 128],
                                             wv_sb[:, ki, n * 512:(n + 1) * 512],
                                             start=(ki == 0), stop=(ki == KH - 1))
                        nc.scalar.copy(vN[:, t, n * 512:(n + 1) * 512], vps[:])

                if debug and c == 0:
                    nc.sync.dma_start(dbg["dqT"].ap(), qT[:])
                    nc.sync.dma_start(dbg["dkT"].ap(), kT[:])
                    nc.sync.dma_start(dbg["dvN"].ap(), vN[:])
                wo_sb = stream_w(wo)
                # ---------- stage 5: attention ----------
                ocat = ws.tile([128, NT, H], bf, tag="hidT_oc_res")
                if debug and c == 0:
                    dsq_sb = sm.tile([128, NT], f32, tag="dsq")
                for h in range(NUM_HEADS):
                    et = ws.tile([128, KH, CHUNK], fp16, tag="xts_et", bufs=2)
                    for kt in range(NT):
                        stp = ps2.tile([128, CHUNK], f32, tag="st")
                        for qn in range(2):
                            for dk in range(2):
                                nc.tensor.matmul(
                                    stp[:, qn * 512:(qn + 1) * 512],
                                    kT[:, 2 * h + dk, kt * 128:(kt + 1) * 128],
                                    qT[:, 2 * h + dk, qn * 512:(qn + 1) * 512],
                                    start=(dk == 0), stop=(dk == 1))
                        # exp(score/sqrt(hd)); values are O(1e-1) so no max-sub needed
                        nc.scalar.activation(et[:, kt, :], stp[:], Act.Exp,
                                             scale=float(HD ** -0.5))
                    for qt in range(NT):
                        ovp = ps.tile([128, 512], f32, tag="mm")
                        for kt in range(NT):
                            # O_unnorm[q, d] accumulation; the extra N=1 matmul
                            # with a ones column gives s[q] = sum_k exp in the
                            # same [q_part, 1] orientation the normalization
                            # needs (same lhsT -> weight load is reused).
                            nc.tensor.matmul(ovp[:, :HD], et[:, kt, qt * 128:(qt + 1) * 128],
                                             vN[:, kt, h * HD:(h + 1) * HD],
                                             start=(kt == 0), stop=(kt == NT - 1))
                            # start=False even at kt==0: start=True clears the
                            # whole PSUM bank and would wipe the V-matmul's
                            # kt==0 contribution.  The bank-clear from the
                            # V-matmul above leaves this column's has_written
                            # bits 0, so kt==0 overwrites (not accumulates).
                            nc.tensor.matmul(ovp[:, HD:HD + 1],
                                             et[:, kt, qt * 128:(qt + 1) * 128],
                                             ones_sb[:],
                                             start=False, stop=(kt == NT - 1),
                                             skip_group_check=True)
                        rq = sm.tile([128, 1], f32, tag="rq")
                        if debug and c == 0 and h == NUM_HEADS - 1:
                            nc.vector.tensor_copy(dsq_sb[:, qt:qt + 1], ovp[:, HD:HD + 1])
                        nc.vector.reciprocal(rq[:], ovp[:, HD:HD + 1])
                        nc.vector.tensor_scalar(ocat[:, qt, h * HD:(h + 1) * HD],
                                                ovp[:, :HD], rq[:], None,
                                                op0=Alu.mult)
                otc = ws.tile([128, KH, CHUNK], bf, tag="qT_otc")
                for qt in range(NT):
                    nc.sync.dma_start_transpose(otc[:, :, qt * 128:(qt + 1) * 128],
                                                ocat[:, qt, :])

                if debug and c == 0:
                    nc.sync.dma_start(dbg["det"].ap(), et[:])
                    nc.sync.dma_start(dbg["docat"].ap(), ocat[:])
                    nc.sync.dma_start(dbg["dsq"].ap(), dsq_sb[:])
                # ---------- stage 6: wo proj + residual + out LN ----------
                res = ws.tile([128, NT, H], bf, tag="hidT_oc_res")
                z = ws.tile([128, NT, H], bf, tag="mixT_z", bufs=2)
                zT = ws.tile([128, KH, CHUNK], bf, tag="kT_zT")
                for t in range(NT):
                    for n in range(2):
                        ops_ = ps.tile([128, 512], f32, tag="mm")
                        for fi in range(KH):
                            nc.tensor.matmul(ops_[:], otc[:, fi, t * 128:(t + 1) * 128],
                                             wo_sb[:, fi, n * 512:(n + 1) * 512],
                                             start=(fi == 0), stop=(fi == KH - 1))
                        nc.vector.tensor_add(res[:, t, n * 512:(n + 1) * 512], ops_[:],
                                             mixN[:, t, n * 512:(n + 1) * 512])
                    st6 = sm.tile([128, 2, 6], f32, tag="st6b")
                    for half in range(2):
                        nc.vector.bn_stats(st6[:, half, :],
                                           res[:, t, half * 512:(half + 1) * 512])
                    mv = sm.tile([128, 2], f32, tag="mv")
                    nc.vector.bn_aggr(mv[:], st6[:])
                    sq = sm.tile([128, 1], f32, tag="sq")
                    nc.scalar.activation(sq[:], mv[:, 1:2], Act.Sqrt, bias=eps_sb[:])
                    iv = sm.tile([128, 1], f32, tag="iv")
                    nc.vector.reciprocal(iv[:], sq[:])
                    nc.vector.tensor_scalar(z[:, t, :], res[:, t, :],
                                            mv[:, 0:1], iv[:],
                                            op0=Alu.subtract, op1=Alu.mult)
                    nc.sync.dma_start_transpose(zT[:, :, t * 128:(t + 1) * 128],
                                                z[:, t, :])

                if debug and c == 0:
                    nc.sync.dma_start(dbg["dres"].ap(), res[:])
                    nc.sync.dma_start(dbg["dz"].ap(), z[:])
                    nc.sync.dma_start(dbg["dzT"].ap(), zT[:])
                # ---------- stage 7: output projection ----------
                ych = ws.tile([128, NT, G], f32, tag="ych", bufs=1)
                for t in range(NT):
                    yps = ps.tile([128, 512], f32, tag="mm")
                    for fi in range(KH):
                        nc.tensor.matmul(yps[:, :G], zT[:, fi, t * 128:(t + 1) * 128],
                                         gw_sb[:, fi, :],
                                         start=(fi == 0), stop=(fi == KH - 1))
                    nc.vector.tensor_add(ych[:, t, :], yps[:, :G], bw_sb[:])
                for hh in range(2):
                    nc.sync.dma_start(
                        y.ap()[c, hh * 512:(hh + 1) * 512, :].rearrange(
                            "(t p) g -> p t g", p=128),
                        ych[:, hh * 4:(hh + 1) * 4, :])

    nc.compile()
    return nc


def _get_compiled():
    global _COMPILED
    if _COMPILED is None:
        _COMPILED = _build()
    return _COMPILED


def _prep_inputs(inputs):
    f32 = np.float32

    def a(name):
        return np.asarray(inputs[name], dtype=f32)

    x = a("x")
    mw = a("mother_wavelets")
    scales = a("scales")
    norm = np.sqrt(np.sum(mw ** 2, axis=2, keepdims=True))
    kern = (mw / np.maximum(norm, 1e-12)) * (1.0 / (1.0 + np.exp(-scales)))
    kern = kern[0, :, :, 0]                      # (W, H)
    kernT = np.ascontiguousarray(kern.T).astype(BF16)

    w1a = np.concatenate([a("mix_w1"), a("mix_b1")[None, :]], axis=0).astype(BF16)
    gln = np.ascontiguousarray(a("mix_ln_g").reshape(KM, 128).T).astype(f32)
    bln = np.ascontiguousarray(a("mix_ln_b").reshape(KM, 128).T).astype(f32)
    w2 = a("mix_w2").astype(BF16)
    b2c = np.ascontiguousarray(a("mix_b2").reshape(KH, 128).T).astype(f32)
    gw = (a("out_ln_g")[:, None] * a("out_w")).astype(BF16)
    bw_vec = a("out_ln_b") @ a("out_w") + a("out_b")
    bw = np.tile(bw_vec[None, :], (128, 1)).astype(f32)

    shared = {
        "kernt": kernT, "w1a": w1a, "gln": gln, "bln": bln, "w2": w2,
        "b2c": b2c, "wq": a("wq").astype(BF16), "wk": a("wk").astype(BF16),
        "wv": a("wv").astype(BF16), "wo": a("wo").astype(BF16),
        "gw": gw, "bw": bw,
    }

    xc = x.reshape(N_CHUNKS, CHUNK, H)
    xt_all = np.ascontiguousarray(xc.transpose(0, 2, 1)).astype(BF16)  # (16, H, CHUNK)
    in_maps = []
    for core in range(N_CORES):
        m = dict(shared)
        m["xt"] = np.ascontiguousarray(xt_all[core * CPC:(core + 1) * CPC])
        in_maps.append(m)
    return in_maps


def kernel(**inputs) -> np.ndarray:
    from concourse.bass_utils import run_bass_kernel_spmd

    nc = _get_compiled()
    in_maps = _prep_inputs(inputs)
    res = run_bass_kernel_spmd(nc, in_maps, core_ids=list(range(N_CORES)))
    out = np.concatenate([r["y"] for r in res.results], axis=0)  # (16, CHUNK, G)
    return out.reshape(B, S, G).astype(np.float32)



# revision 22
# speedup vs baseline: 1.2358x; 1.2358x over previous
"""Trainium2 Bass kernel for nn_EntropyLM (wavelet-coeff mixer + chunked MHA + output proj).

Strategy: data-parallel over the 16 independent (batch x chunk) blocks, 2 per
NeuronCore.  The numerically-critical path (wavelet coeffs, mixer, residual
stream, output projection) runs in fp16 on the PE (same speed as bf16, 8x the
mantissa); the error-tolerant bulk (q/k/v projections, attention scores, PV,
attention-out projection) runs in fp8 e4m3 with DoubleRow perf mode, which
contracts K=256 per instruction at 0.5 cycles/row -- 4x bf16 matmul
throughput in the HW cost model.

Per-tensor power-of-two scales keep fp8 operands in [~1, 200]; all scale
corrections are folded into PSUM-evacuation ops that are needed anyway.

Layouts per chunk (CHUNK=1024 tokens, H=1024 features):
  * "T" tensors are feature-major [feat_part, ktile, token]; "N" tensors are
    token-major [token_part, ttile, feat].
  * Attention-out (ocat, token-major fp8) is transposed for the wo matmul by
    viewing fp8 pairs as uint16 through the DMA xbar transpose; the row
    permutation this induces on the contraction index is compensated by
    pre-permuting wo's rows on the host (wo8p).
  * The softmax denominator comes from a 1-column DoubleRow matmul against a
    constant 0.125 vector (reusing the PV lhsT weights); normalization is a
    per-partition scale on the PV evacuation.

The two chunks per core are software-pipelined by emission order: chunk B's
PE-heavy projection tiles are drained as filler between chunk A's Act-bound
attention pieces so the PE never idles waiting on exp().
"""

import numpy as np
import ml_dtypes

B, S, H, G, W = 4, 4096, 1024, 256, 8
CHUNK = 1024
NUM_HEADS = 4
HD = H // NUM_HEADS          # 256 per-head dim
HM = H // 2                  # 512 mixer hidden
N_CHUNKS = B * (S // CHUNK)  # 16 independent chunks
N_CORES = 8
CPC = N_CHUNKS // N_CORES    # 2 chunks per core
NT = CHUNK // 128            # 8 token tiles
KH = H // 128                # 8 feature tiles (H)
KM = HM // 128               # 4 feature tiles (HM)
EPS = 1e-5
BF16 = ml_dtypes.bfloat16
F8 = ml_dtypes.float8_e4m3
F16 = np.float16

# fp8 scales (powers of two; folded into evacuation ops)
S_W8 = 1024.0    # wq/wk/wv/wo weight scale
S_M8 = 64.0      # mix8 activation scale
S_Q8 = 128.0     # q/k fp8 scale
S_V8 = 128.0     # v fp8 scale
S_ET = 16.0      # exp(score) scale
C_ONE = 0.5      # denominator ones value -> ocat = (S_V8/C_ONE) * o = 256*o
S_O8 = S_V8 / C_ONE              # 1024
INV_WO = 1.0 / (S_O8 * S_W8)     # 2^-20

_COMPILED = None


def _build(debug=False):
    import concourse.bass as bass  # noqa: F401
    import concourse.tile as tile
    from concourse import bacc, mybir

    f8 = mybir.dt.float8e4
    u16 = mybir.dt.uint16
    fp16 = mybir.dt.float16
    f32 = mybir.dt.float32
    Alu = mybir.AluOpType
    Act = mybir.ActivationFunctionType
    DR = mybir.MatmulPerfMode.DoubleRow

    nc = bacc.Bacc("TRN2", target_bir_lowering=False, debug=False,
                   enable_asserts=True, num_devices=N_CORES)

    # ---- DRAM tensors (per-core views; same NEFF on all 8 cores) ----
    xt = nc.dram_tensor("xt", [CPC, H, CHUNK], fp16, kind="ExternalInput")
    kernT = nc.dram_tensor("kernt", [H, W], fp16, kind="ExternalInput")
    w1a = nc.dram_tensor("w1a", [W + 1, HM], fp16, kind="ExternalInput")
    gln = nc.dram_tensor("gln", [128, KM], f32, kind="ExternalInput")
    bln = nc.dram_tensor("bln", [128, KM], f32, kind="ExternalInput")
    w2 = nc.dram_tensor("w2", [HM, H], fp16, kind="ExternalInput")
    b2c = nc.dram_tensor("b2c", [128, KH], f32, kind="ExternalInput")
    wq8 = nc.dram_tensor("wq8", [H, H], f8, kind="ExternalInput")
    wk8 = nc.dram_tensor("wk8", [H, H], f8, kind="ExternalInput")
    wv8 = nc.dram_tensor("wv8", [H, H], f8, kind="ExternalInput")
    wo8 = nc.dram_tensor("wo8", [H, H], f8, kind="ExternalInput")
    gw = nc.dram_tensor("gw", [H, G], fp16, kind="ExternalInput")
    bw = nc.dram_tensor("bw", [128, G], f32, kind="ExternalInput")
    y = nc.dram_tensor("y", [CPC, CHUNK, G], f32, kind="ExternalOutput")
    dbg = {}
    if debug:
        for nm, shp, dt in [
            ("dcoef", [W + 1, CHUNK], fp16),
            ("dhidT", [128, KM, CHUNK], fp16),
            ("dmix8", [128, KH, CHUNK], f8),
            ("dmixN", [128, NT, H], fp16),
            ("dqT", [128, KH, CHUNK], f8),
            ("dkT", [128, KH, CHUNK], f8),
            ("dvN", [128, NT, H], f8),
            ("det", [128, KH, CHUNK], f8),
            ("ddn", [128, NUM_HEADS, NT], f32),
            ("dotc", [128, KH, CHUNK], f8),
            ("dres", [128, NT, H], fp16),
        ]:
            dbg[nm] = nc.dram_tensor(nm, shp, dt, kind="ExternalOutput")

    with tile.TileContext(nc) as tc:
        with (
            tc.tile_pool(name="wp", bufs=1) as wp,
            tc.tile_pool(name="ws", bufs=1) as ws,
            tc.tile_pool(name="sm", bufs=2) as sm,
            tc.tile_pool(name="ps", bufs=1, space="PSUM") as ps,
        ):
            # ---------- persistent weights ----------
            kt_sb = wp.tile([128, KH, W], fp16, tag="ktw")
            nc.sync.dma_start(kt_sb[:], kernT.ap().rearrange("(i p) w -> p i w", p=128))
            w1a_sb = wp.tile([W + 1, HM], fp16, tag="w1a")
            nc.sync.dma_start(w1a_sb[:], w1a.ap())
            gln_sb = wp.tile([128, KM], f32, tag="gln")
            nc.sync.dma_start(gln_sb[:], gln.ap())
            bln_sb = wp.tile([128, KM], f32, tag="bln")
            nc.sync.dma_start(bln_sb[:], bln.ap())
            b2_sb = wp.tile([128, KH], f32, tag="b2")
            nc.sync.dma_start(b2_sb[:], b2c.ap())
            w2_sb = wp.tile([128, KM, H], fp16, tag="w2s")
            nc.sync.dma_start(w2_sb[:], w2.ap().rearrange("(i p) m -> p i m", p=128))
            wq_sb = wp.tile([128, KH, H], f8, tag="wq")
            nc.sync.dma_start(wq_sb[:], wq8.ap().rearrange("(i p) m -> p i m", p=128))
            wk_sb = wp.tile([128, KH, H], f8, tag="wk")
            nc.sync.dma_start(wk_sb[:], wk8.ap().rearrange("(i p) m -> p i m", p=128))
            wv_sb = wp.tile([128, KH, H], f8, tag="wv")
            nc.sync.dma_start(wv_sb[:], wv8.ap().rearrange("(i p) m -> p i m", p=128))
            wo_sb = wp.tile([128, KH, H], f8, tag="wo")
            nc.sync.dma_start(wo_sb[:], wo8.ap().rearrange("(i p) m -> p i m", p=128))
            gw_sb = wp.tile([128, KH, G], fp16, tag="gw")
            nc.sync.dma_start(gw_sb[:], gw.ap().rearrange("(i p) g -> p i g", p=128))
            bw_sb = wp.tile([128, G], f32, tag="bw")
            nc.sync.dma_start(bw_sb[:], bw.ap())
            ones8 = wp.tile([128, 2, 1], f8, tag="ones")
            nc.vector.memset(ones8[:], C_ONE)
            eps_sb = wp.tile([128, 1], f32, tag="eps")
            nc.vector.memset(eps_sb[:], EPS)
            lns_sb = wp.tile([128, 1], f32, tag="lns")
            nc.vector.memset(lns_sb[:], float(np.log(S_ET)))

            # ---------- per-chunk state ----------
            st = [dict() for _ in range(CPC)]

            def psum_big(n=1024):
                return ps.tile([128, n], f32, tag="big", bufs=2, name="pbig")

            def psum_st():
                return ps.tile([128, 1024], f32, tag="st", bufs=1, name="pst")

            def psum_pv():
                return ps.tile([128, 256], f32, tag="pv", bufs=2, name="ppv")

            # ----- S0+S1: stream x, wavelet coeffs -----
            def s01(c):
                coef = ws.tile([W + 1, CHUNK], fp16, tag=f"coef{c}")
                nc.gpsimd.memset(coef[:, :], 1.0)
                cps = [psum_big(), psum_big()]
                for ki in range(KH):
                    xs = ws.tile([128, CHUNK], fp16, tag=f"xs{c}", bufs=2,
                                 name="xs")
                    nc.sync.dma_start(
                        xs[:], xt.ap()[c, ki * 128:(ki + 1) * 128, :])
                    for n in range(2):
                        nc.tensor.matmul(
                            cps[n][:W, :512], kt_sb[:, ki, :],
                            xs[:, n * 512:(n + 1) * 512],
                            start=(ki == 0), stop=(ki == KH - 1))
                for n in range(2):
                    nc.scalar.copy(coef[:W, n * 512:(n + 1) * 512], cps[n][:W, :512])
                st[c]["coef"] = coef

            # ----- S2: mixer hidden + LN + gelu -> hidT (list of 9 thunks) -----
            def s2_tiles(c):
                coef = st[c]["coef"]
                hidT = ws.tile([128, KM, CHUNK], fp16, tag=f"hvy{c}")
                st[c]["hidT"] = hidT

                def tile_t(t):
                    hps = psum_big(512)
                    nc.tensor.matmul(hps[:, :512], coef[:, t * 128:(t + 1) * 128],
                                     w1a_sb[:], start=True, stop=True)
                    st6 = sm.tile([128, 6], f32, tag="st6")
                    nc.vector.bn_stats(st6[:], hps[:, :512])
                    mv = sm.tile([128, 2], f32, tag="mv")
                    nc.vector.bn_aggr(mv[:], st6[:])
                    sq = sm.tile([128, 1], f32, tag="sq")
                    nc.scalar.activation(sq[:], mv[:, 1:2], Act.Sqrt, bias=eps_sb[:])
                    iv = sm.tile([128, 1], f32, tag="iv")
                    nc.vector.reciprocal_approx_fast(iv[:], sq[:])
                    tmp = sm.tile([128, 512], fp16, tag="ntmp", bufs=2)
                    nc.vector.tensor_scalar(tmp[:], hps[:, :512],
                                            mv[:, 0:1], iv[:],
                                            op0=Alu.subtract, op1=Alu.mult)
                    nc.sync.dma_start_transpose(hidT[:, :, t * 128:(t + 1) * 128],
                                                tmp[:])

                def fin():
                    for ki in range(KM):
                        sl = hidT[:, ki, :]
                        nc.gpsimd.tensor_scalar(sl, sl,
                                                gln_sb[:, ki:ki + 1],
                                                bln_sb[:, ki:ki + 1],
                                                op0=Alu.mult, op1=Alu.add)
                        nc.scalar.activation(sl, sl, Act.Gelu)
                    if debug and c == 0:
                        nc.sync.dma_start(dbg["dhidT"].ap(), hidT[:])
                        nc.sync.dma_start(dbg["dcoef"].ap(), coef[:])

                return [lambda t=t: tile_t(t) for t in range(NT)] + [fin]

            # ----- S3: mixed (fp16 matmul) -> mix8 + mixN (staged transpose) --
            def s3_tiles(c):
                hidT = st[c]["hidT"]
                mix8 = ws.tile([128, KH, CHUNK], f8, tag=f"m8{c}")
                mixN = ws.tile([128, NT, H], fp16, tag=f"mN{c}")
                st[c]["mix8"] = mix8
                st[c]["mixN"] = mixN

                def tile_m(m):
                    mps = psum_big()
                    for n in range(2):
                        for ki in range(KM):
                            nc.tensor.matmul(mps[:, n * 512:(n + 1) * 512],
                                             w2_sb[:, ki, m * 128:(m + 1) * 128],
                                             hidT[:, ki, n * 512:(n + 1) * 512],
                                             start=(ki == 0), stop=(ki == KM - 1))
                    mt = sm.tile([128, CHUNK], fp16, tag="mt", bufs=3)
                    nc.scalar.activation(mt[:], mps[:], Act.Identity,
                                         bias=b2_sb[:, m:m + 1])
                    nc.vector.tensor_scalar(mix8[:, m, :], mps[:],
                                            b2_sb[:, m:m + 1], S_M8,
                                            op0=Alu.add, op1=Alu.mult)
                    nc.sync.dma_start_transpose(mixN[:, :, m * 128:(m + 1) * 128],
                                                mt[:])

                def fin():
                    if debug and c == 0:
                        nc.sync.dma_start(dbg["dmix8"].ap(), mix8[:])
                        nc.sync.dma_start(dbg["dmixN"].ap(), mixN[:])

                return [lambda m=m: tile_m(m) for m in range(KH)] + [fin]

            # ----- S4: q/k/v projections (fp8 DoubleRow) -----
            def s4_tiles(c):
                mix8 = st[c]["mix8"]
                qT = ws.tile([128, KH, CHUNK], f8, tag=f"q8{c}")
                kT = ws.tile([128, KH, CHUNK], f8, tag=f"k8{c}")
                vN = ws.tile([128, NT, H], f8, tag=f"hvy{c}")
                st[c]["qT"] = qT
                st[c]["kT"] = kT
                st[c]["vN"] = vN

                def proj_m(dst, wsb, m, on_vec):
                    qps = psum_big()
                    for n in range(2):
                        for g in range(4):
                            nc.tensor.matmul(
                                qps[:, n * 512:(n + 1) * 512],
                                wsb[:, 2 * g:2 * g + 2, m * 128:(m + 1) * 128],
                                mix8[:, 2 * g:2 * g + 2, n * 512:(n + 1) * 512],
                                start=(g == 0), stop=(g == 3), perf_mode=DR)
                    sc = S_Q8 / (S_M8 * S_W8)
                    if on_vec:
                        nc.vector.tensor_scalar(dst[:, m, :], qps[:], sc, None,
                                                op0=Alu.mult)
                    else:
                        nc.scalar.activation(dst[:, m, :], qps[:], Act.Copy,
                                             scale=sc)

                def v_t(t):
                    vps = psum_big()
                    for n in range(2):
                        for g in range(4):
                            nc.tensor.matmul(
                                vps[:, n * 512:(n + 1) * 512],
                                mix8[:, 2 * g:2 * g + 2, t * 128:(t + 1) * 128],
                                wv_sb[:, 2 * g:2 * g + 2, n * 512:(n + 1) * 512],
                                start=(g == 0), stop=(g == 3), perf_mode=DR)
                    nc.vector.tensor_scalar(vN[:, t, :], vps[:],
                                            S_V8 / (S_M8 * S_W8), None,
                                            op0=Alu.mult)

                thunks = []
                for m in range(KH):
                    thunks.append(lambda m=m: proj_m(qT, wq_sb, m, False))
                for m in range(KH):
                    thunks.append(lambda m=m: proj_m(kT, wk_sb, m, True))
                for t in range(NT):
                    thunks.append(lambda t=t: v_t(t))

                def fin():
                    if debug and c == 0:
                        nc.sync.dma_start(dbg["dqT"].ap(), qT[:])
                        nc.sync.dma_start(dbg["dkT"].ap(), kT[:])
                        nc.sync.dma_start(dbg["dvN"].ap(), vN[:])
                thunks.append(fin)
                return thunks

            # ----- S5: attention per head (scores -> exp -> PV+denom -> ocat) --
            def s5_head(c, h, drain):
                qT, kT, vN = st[c]["qT"], st[c]["kT"], st[c]["vN"]
                if h == 0:
                    st[c]["ocat"] = ws.tile([128, NT, HD], fp16,
                                            tag=f"oc{c}", name="ocat")
                    st[c]["et"] = ws.tile([128, KH, CHUNK], f8,
                                          tag=f"et{c}", name="et")
                    st[c]["otc"] = ws.tile([128, KH, CHUNK], f8,
                                           tag=f"m8{c}", name="otc")
                ocat = st[c]["ocat"]
                otc = st[c]["otc"]
                et = st[c]["et"]
                exp_scale = float(HD ** -0.5) / (S_Q8 * S_Q8)

                for kt in range(NT):
                    stp = psum_st()
                    for qn in range(2):
                        nc.tensor.matmul(
                            stp[:, qn * 512:(qn + 1) * 512],
                            kT[:, 2 * h:2 * h + 2, kt * 128:(kt + 1) * 128],
                            qT[:, 2 * h:2 * h + 2, qn * 512:(qn + 1) * 512],
                            start=True, stop=True, perf_mode=DR)
                    nc.scalar.activation(et[:, kt, :], stp[:], Act.Exp,
                                         scale=exp_scale, bias=lns_sb[:])
                    drain(1)
                if debug and c == 0 and h == NUM_HEADS - 1:
                    nc.sync.dma_start(dbg["det"].ap(), et[:])
                # denominator: d[q] = sum_k et * C_ONE, one column per qt
                dps = psum_st()
                for qt in range(NT):
                    for g in range(4):
                        nc.tensor.matmul(
                            dps[:, qt:qt + 1],
                            et[:, 2 * g:2 * g + 2, qt * 128:(qt + 1) * 128],
                            ones8[:], start=(qt == 0 and g == 0), stop=(g == 3),
                            perf_mode=DR, skip_group_check=True)
                rq = sm.tile([128, NT], f32, tag="rq", bufs=2)
                nc.vector.reciprocal_approx_fast(rq[:], dps[:, :NT])
                if debug and c == 0:
                    nc.sync.dma_start(dbg["ddn"].ap()[:, h, :], rq[:])
                drain(1)
                for qt in range(NT):
                    pvp = psum_pv()
                    for g in range(4):
                        nc.tensor.matmul(
                            pvp[:],
                            et[:, 2 * g:2 * g + 2, qt * 128:(qt + 1) * 128],
                            vN[:, 2 * g:2 * g + 2, h * HD:(h + 1) * HD],
                            start=(g == 0), stop=(g == 3), perf_mode=DR)
                    dst = ocat[:, qt, :]
                    nc.vector.tensor_scalar(dst, pvp[:], rq[:, qt:qt + 1],
                                            None, op0=Alu.mult)
                    if qt % 3 == 2:
                        drain(1)
                # transpose this head's output into the fp8 feature-major otc
                for qt in range(NT):
                    ot = sm.tile([128, 2, 128], fp16, tag="ott", bufs=2)
                    nc.sync.dma_start_transpose(ot[:], ocat[:, qt, :])
                    nc.gpsimd.tensor_copy(
                        otc[:, 2 * h:2 * h + 2, qt * 128:(qt + 1) * 128], ot[:])
                    if qt % 4 == 3:
                        drain(1)

            def s5_fin(c):
                if debug and c == 0:
                    nc.sync.dma_start(dbg["dotc"].ap(), st[c]["otc"][:])

            # ----- S6+S7: wo proj + residual + LN + output proj, per t ------
            def s67_tiles(c):
                otc8 = st[c]["otc"]  # [128, KH, CHUNK] fp8
                mixN = st[c]["mixN"]
                ych = ws.tile([128, NT, G], f32, tag=f"hvy{c}", name="ych")

                def tile_t(t):
                    ops_ = psum_big()
                    for n in range(2):
                        for g in range(4):
                            nc.tensor.matmul(
                                ops_[:, n * 512:(n + 1) * 512],
                                otc8[:, 2 * g:2 * g + 2, t * 128:(t + 1) * 128],
                                wo_sb[:, 2 * g:2 * g + 2, n * 512:(n + 1) * 512],
                                start=(g == 0), stop=(g == 3), perf_mode=DR)
                    res = mixN[:, t, :]
                    nc.vector.scalar_tensor_tensor(res, ops_[:], INV_WO, res,
                                                   op0=Alu.mult, op1=Alu.add)
                    st6 = sm.tile([128, 2, 6], f32, tag="st6b")
                    for half in range(2):
                        nc.vector.bn_stats(st6[:, half, :],
                                           mixN[:, t, half * 512:(half + 1) * 512])
                    mv = sm.tile([128, 2], f32, tag="mvb")
                    nc.vector.bn_aggr(mv[:], st6[:])
                    sq = sm.tile([128, 1], f32, tag="sqb")
                    nc.scalar.activation(sq[:], mv[:, 1:2], Act.Sqrt, bias=eps_sb[:])
                    iv = sm.tile([128, 1], f32, tag="ivb")
                    nc.vector.reciprocal_approx_fast(iv[:], sq[:])
                    nc.gpsimd.tensor_scalar(res, res, mv[:, 0:1], iv[:],
                                            op0=Alu.subtract, op1=Alu.mult)
                    zst = sm.tile([128, KH, 128], fp16, tag="zst", bufs=2)
                    nc.sync.dma_start_transpose(zst[:], res)
                    yps = psum_pv()
                    for i in range(KH):
                        nc.tensor.matmul(yps[:], zst[:, i, :],
                                         gw_sb[:, i, :],
                                         start=(i == 0), stop=(i == KH - 1))
                    nc.vector.tensor_tensor(ych[:, t, :], yps[:], bw_sb[:],
                                            op=Alu.add)
                    if debug and c == 0 and t == NT - 1:
                        nc.sync.dma_start(dbg["dres"].ap(), mixN[:])

                def fin():
                    for hh in range(2):
                        nc.sync.dma_start(
                            y.ap()[c, hh * 512:(hh + 1) * 512, :].rearrange(
                                "(t p) g -> p t g", p=128),
                            ych[:, hh * 4:(hh + 1) * 4, :])

                return [lambda t=t: tile_t(t) for t in range(NT)] + [fin]

            # ---------------- schedule ----------------
            from collections import deque

            def mk_drain(q):
                def drain(n):
                    for _ in range(min(n, len(q))):
                        q.popleft()()
                return drain

            a, b = 0, 1
            s01(a)
            s01(b)
            qa = deque(s2_tiles(a))
            while qa:
                qa.popleft()()
            # A.S3 with B.S2 interleaved
            qb = deque(s2_tiles(b))
            for i, th in enumerate(s3_tiles(a)):
                th()
                if qb:
                    qb.popleft()()
            while qb:
                qb.popleft()()
            # A.S4 solid
            for th in s4_tiles(a):
                th()
            # A.S5 with B.S3+S4 as filler
            qb = deque(s3_tiles(b) + s4_tiles(b))
            drain_b = mk_drain(qb)
            for h in range(NUM_HEADS):
                s5_head(a, h, drain_b)
            while qb:
                qb.popleft()()
            s5_fin(a)
            # B.S5 with A.S6+S7 as filler
            qa = deque(s67_tiles(a))
            drain_a = mk_drain(qa)
            for h in range(NUM_HEADS):
                s5_head(b, h, drain_a)
            while qa:
                qa.popleft()()
            s5_fin(b)
            # B tail
            for th in s67_tiles(b):
                th()

    nc.compile()
    return nc


def _get_compiled():
    global _COMPILED
    if _COMPILED is None:
        _COMPILED = _build()
    return _COMPILED


def _prep_inputs(inputs):
    f32 = np.float32

    def a(name):
        return np.asarray(inputs[name], dtype=f32)

    x = a("x")
    mw = a("mother_wavelets")
    scales = a("scales")
    norm = np.sqrt(np.sum(mw ** 2, axis=2, keepdims=True))
    kern = (mw / np.maximum(norm, 1e-12)) * (1.0 / (1.0 + np.exp(-scales)))
    kern = kern[0, :, :, 0]                      # (W, H)
    kernT = np.ascontiguousarray(kern.T).astype(F16)

    w1a = np.concatenate([a("mix_w1"), a("mix_b1")[None, :]], axis=0).astype(F16)
    gln = np.ascontiguousarray(a("mix_ln_g").reshape(KM, 128).T).astype(f32)
    bln = np.ascontiguousarray(a("mix_ln_b").reshape(KM, 128).T).astype(f32)
    w2 = a("mix_w2").astype(F16)
    b2c = np.ascontiguousarray(a("mix_b2").reshape(KH, 128).T).astype(f32)
    gw = (a("out_ln_g")[:, None] * a("out_w")).astype(F16)
    bw_vec = a("out_ln_b") @ a("out_w") + a("out_b")
    bw = np.tile(bw_vec[None, :], (128, 1)).astype(f32)

    def to8(w, s):
        ws = w * s
        am = np.abs(ws).max()
        assert am < 224.0, f"fp8 overflow: {am}"
        return ws.astype(F8)

    wq = to8(a("wq"), S_W8)
    wk = to8(a("wk"), S_W8)
    wv = to8(a("wv"), S_W8)
    wo = to8(a("wo"), S_W8)

    shared = {
        "kernt": kernT, "w1a": w1a, "gln": gln, "bln": bln, "w2": w2,
        "b2c": b2c, "wq8": wq, "wk8": wk, "wv8": wv, "wo8": wo,
        "gw": gw, "bw": bw,
    }

    xc = x.reshape(N_CHUNKS, CHUNK, H)
    xt_all = np.ascontiguousarray(xc.transpose(0, 2, 1)).astype(F16)
    in_maps = []
    for core in range(N_CORES):
        m = dict(shared)
        m["xt"] = np.ascontiguousarray(xt_all[core * CPC:(core + 1) * CPC])
        in_maps.append(m)
    return in_maps


def kernel(**inputs) -> np.ndarray:
    from concourse.bass_utils import run_bass_kernel_spmd

    nc = _get_compiled()
    in_maps = _prep_inputs(inputs)
    res = run_bass_kernel_spmd(nc, in_maps, core_ids=list(range(N_CORES)))
    out = np.concatenate([r["y"] for r in res.results], axis=0)  # (16, CHUNK, G)
    return out.reshape(B, S, G).astype(np.float32)


# revision 36
# speedup vs baseline: 1.3916x; 1.1260x over previous
"""Trainium2 Bass kernel for nn_EntropyLM (wavelet-coeff mixer + chunked MHA + output proj).

Strategy: data-parallel over the 16 independent (batch x chunk) blocks, 2 per
NeuronCore.  The numerically-critical path (wavelet coeffs, mixer, residual
stream, output projection) runs in fp16 on the PE (same speed as bf16, 8x the
mantissa); the error-tolerant bulk (q/k/v projections, attention scores, PV,
attention-out projection) runs in fp8 e4m3 with DoubleRow perf mode, which
contracts K=256 per instruction at 0.5 cycles/row -- 4x bf16 matmul
throughput in the HW cost model.

Per-tensor power-of-two scales keep fp8 operands in [~1, 200]; all scale
corrections are folded into PSUM-evacuation ops that are needed anyway.

Layouts per chunk (CHUNK=1024 tokens, H=1024 features):
  * "T" tensors are feature-major [feat_part, ktile, token]; "N" tensors are
    token-major [token_part, ttile, feat].
  * Attention-out (ocat, token-major fp8) is transposed for the wo matmul by
    viewing fp8 pairs as uint16 through the DMA xbar transpose; the row
    permutation this induces on the contraction index is compensated by
    pre-permuting wo's rows on the host (wo8p).
  * The softmax denominator comes from a 1-column DoubleRow matmul against a
    constant 0.125 vector (reusing the PV lhsT weights); normalization is a
    per-partition scale on the PV evacuation.

The two chunks per core are software-pipelined by emission order: chunk B's
PE-heavy projection tiles are drained as filler between chunk A's Act-bound
attention pieces so the PE never idles waiting on exp().
"""

import numpy as np
import ml_dtypes

B, S, H, G, W = 4, 4096, 1024, 256, 8
CHUNK = 1024
NUM_HEADS = 4
HD = H // NUM_HEADS          # 256 per-head dim
HM = H // 2                  # 512 mixer hidden
N_CHUNKS = B * (S // CHUNK)  # 16 independent chunks
N_CORES = 8
CPC = N_CHUNKS // N_CORES    # 2 chunks per core
NT = CHUNK // 128            # 8 token tiles
KH = H // 128                # 8 feature tiles (H)
KM = HM // 128               # 4 feature tiles (HM)
EPS = 1e-5
BF16 = ml_dtypes.bfloat16
F8 = ml_dtypes.float8_e4m3
F16 = np.float16

# fp8 scales (powers of two; folded into evacuation ops)
S_W8 = 1024.0    # wq/wk/wv/wo weight scale
S_M8 = 64.0      # mix8 activation scale
S_Q8 = 128.0     # q/k fp8 scale
S_V8 = 128.0     # v fp8 scale
S_ET = 16.0      # exp(score) scale
C_ONE = 0.5      # denominator ones value -> ocat = (S_V8/C_ONE) * o = 256*o
S_O8 = S_V8 / C_ONE              # 1024
INV_WO = 1.0 / (S_O8 * S_W8)     # 2^-20

_COMPILED = None
STAGE_MARKS = []


def _build(debug=False):
    import concourse.bass as bass  # noqa: F401
    import concourse.tile as tile
    from concourse import bacc, mybir

    f8 = mybir.dt.float8e4
    u16 = mybir.dt.uint16
    fp16 = mybir.dt.float16
    f32 = mybir.dt.float32
    Alu = mybir.AluOpType
    Act = mybir.ActivationFunctionType
    DR = mybir.MatmulPerfMode.DoubleRow

    nc = bacc.Bacc("TRN2", target_bir_lowering=False, debug=False,
                   enable_asserts=True, num_devices=N_CORES)

    # ---- DRAM tensors (per-core views; same NEFF on all 8 cores) ----
    xt = nc.dram_tensor("xt", [CPC, H, CHUNK], fp16, kind="ExternalInput")
    kernT = nc.dram_tensor("kernt", [H, W], fp16, kind="ExternalInput")
    w1a = nc.dram_tensor("w1a", [W + 1, HM], fp16, kind="ExternalInput")
    gln = nc.dram_tensor("gln", [128, KM], f32, kind="ExternalInput")
    bln = nc.dram_tensor("bln", [128, KM], f32, kind="ExternalInput")
    w2 = nc.dram_tensor("w2", [HM, H], fp16, kind="ExternalInput")
    b2c = nc.dram_tensor("b2c", [128, KH], f32, kind="ExternalInput")
    wq8 = nc.dram_tensor("wq8", [H, H], f8, kind="ExternalInput")
    wk8 = nc.dram_tensor("wk8", [H, H], f8, kind="ExternalInput")
    wv8 = nc.dram_tensor("wv8", [H, H], f8, kind="ExternalInput")
    wo8 = nc.dram_tensor("wo8", [H, H], f8, kind="ExternalInput")
    gw = nc.dram_tensor("gw", [H, G], fp16, kind="ExternalInput")
    bw = nc.dram_tensor("bw", [128, G], f32, kind="ExternalInput")
    y = nc.dram_tensor("y", [CPC, CHUNK, G], f32, kind="ExternalOutput")
    dbg = {}
    if debug:
        for nm, shp, dt in [
            ("dcoef", [W + 1, CHUNK], fp16),
            ("dhidT", [128, KM, CHUNK], fp16),
            ("dmix8", [128, KH, CHUNK], f8),
            ("dmixN", [128, NT, H], fp16),
            ("dqT", [128, KH, CHUNK], f8),
            ("dkT", [128, KH, CHUNK], f8),
            ("dvN", [128, NT, H], f8),
            ("det", [128, KH, CHUNK], f8),
            ("ddn", [128, NUM_HEADS, NT], f32),
            ("dotc", [128, KH, CHUNK], f8),
            ("dres", [128, NT, H], fp16),
        ]:
            dbg[nm] = nc.dram_tensor(nm, shp, dt, kind="ExternalOutput")

    with tile.TileContext(nc) as tc:
        with (
            tc.tile_pool(name="wp", bufs=1) as wp,
            tc.tile_pool(name="ws", bufs=1) as ws,
            tc.tile_pool(name="sm", bufs=2) as sm,
            tc.tile_pool(name="ps", bufs=1, space="PSUM") as ps,
        ):
            # ---------- persistent weights ----------
            kt_sb = wp.tile([128, KH, W], fp16, tag="ktw")
            nc.sync.dma_start(kt_sb[:], kernT.ap().rearrange("(i p) w -> p i w", p=128))
            w1a_sb = wp.tile([W + 1, HM], fp16, tag="w1a")
            nc.sync.dma_start(w1a_sb[:], w1a.ap())
            gln_sb = wp.tile([128, KM], f32, tag="gln")
            nc.sync.dma_start(gln_sb[:], gln.ap())
            bln_sb = wp.tile([128, KM], f32, tag="bln")
            nc.sync.dma_start(bln_sb[:], bln.ap())
            b2_sb = wp.tile([128, KH], f32, tag="b2")
            nc.sync.dma_start(b2_sb[:], b2c.ap())
            w2_sb = wp.tile([128, KM, H], fp16, tag="w2s")
            wq_sb = wp.tile([128, KH, H], f8, tag="wq")
            wk_sb = wp.tile([128, KH, H], f8, tag="wk")
            wv_sb = wp.tile([128, KH, H], f8, tag="wv")
            wo_sb = wp.tile([128, KH, H], f8, tag="wo")
            gw_sb = wp.tile([128, KH, G], fp16, tag="gw")
            bw_sb = wp.tile([128, G], f32, tag="bw")

            def load_big_weights():
                # emitted after the x-stream DMAs so they don't delay S1
                nc.sync.dma_start(w2_sb[:],
                                  w2.ap().rearrange("(i p) m -> p i m", p=128))
                nc.scalar.dma_start(wq_sb[:],
                                    wq8.ap().rearrange("(i p) m -> p i m", p=128))
                nc.sync.dma_start(wk_sb[:],
                                  wk8.ap().rearrange("(i p) m -> p i m", p=128))
                nc.scalar.dma_start(wv_sb[:],
                                    wv8.ap().rearrange("(i p) m -> p i m", p=128))
                nc.sync.dma_start(gw_sb[:],
                                  gw.ap().rearrange("(i p) g -> p i g", p=128))
                nc.scalar.dma_start(bw_sb[:], bw.ap())
                nc.scalar.dma_start(wo_sb[:],
                                    wo8.ap().rearrange("(i p) m -> p i m", p=128))
            ones8 = wp.tile([128, 2, 1], f8, tag="ones")
            nc.vector.memset(ones8[:], C_ONE)
            eps_sb = wp.tile([128, 1], f32, tag="eps")
            nc.vector.memset(eps_sb[:], EPS)
            lns_sb = wp.tile([128, 1], f32, tag="lns")
            nc.vector.memset(lns_sb[:], float(np.log(S_ET)))

            # ---------- per-chunk state ----------
            st = [dict() for _ in range(CPC)]

            def psum_big(n=1024):
                return ps.tile([128, n], f32, tag="big", bufs=2, name="pbig")

            def psum_st():
                return ps.tile([128, 1024], f32, tag="st", bufs=2, name="pst")

            # ----- S0+S1: stream x (both queues), wavelet coeffs -----
            def s01_load(c):
                xf = ws.tile([128, KH, CHUNK], fp16, tag=f"xet{c}", name="xf")
                for ki in range(KH):
                    eng = nc.sync if ki % 2 == 0 else nc.scalar
                    eng.dma_start(xf[:, ki, :],
                                  xt.ap()[c, ki * 128:(ki + 1) * 128, :])
                st[c]["xs"] = xf

            def s01_mm(c):
                coef = ws.tile([W + 1, CHUNK], fp16, tag=f"coef{c}")
                nc.gpsimd.memset(coef[:, :], 1.0)
                cps = [psum_big(), psum_big()]
                xf = st[c]["xs"]
                for ki in range(KH):
                    for n in range(2):
                        nc.tensor.matmul(
                            cps[n][:W, :512], kt_sb[:, ki, :],
                            xf[:, ki, n * 512:(n + 1) * 512],
                            start=(ki == 0), stop=(ki == KH - 1))
                for n in range(2):
                    nc.scalar.copy(coef[:W, n * 512:(n + 1) * 512], cps[n][:W, :512])
                st[c]["coef"] = coef

            # ----- S2: mixer hidden + LN + gelu -> hidT (two-pass LN) -------
            def s2_tiles(c):
                coef = st[c]["coef"]
                hidT = ws.tile([128, KM, CHUNK], fp16, tag=f"hvy{c}")
                st[c]["hidT"] = hidT
                mva = sm.tile([128, NT, 2], f32, tag="mva2", bufs=2, name="mva")
                iva = sm.tile([128, NT], f32, tag="iva2", bufs=2, name="iva")
                hps_l = [None] * NT

                def stats_t(t):
                    hps = psum_big(512)
                    hps_l[t] = hps
                    nc.tensor.matmul(hps[:, :512], coef[:, t * 128:(t + 1) * 128],
                                     w1a_sb[:], start=True, stop=True)
                    st6 = sm.tile([128, 6], f32, tag="st6")
                    nc.vector.bn_stats(st6[:], hps[:, :512])
                    nc.vector.bn_aggr(mva[:, t, :], st6[:])
                    tmp = sm.tile([128, 512], fp16, tag="ntmp", bufs=4)
                    nc.vector.tensor_scalar(tmp[:], hps[:, :512],
                                            mva[:, t, 0:1], None,
                                            op0=Alu.subtract)
                    hps_l[t] = tmp

                def half_iv(hh):
                    sq = sm.tile([128, 4], f32, tag="sq2", name="sq2")
                    nc.scalar.activation(sq[:], mva[:, hh * 4:(hh + 1) * 4, 1],
                                         Act.Sqrt, bias=eps_sb[:])
                    nc.vector.reciprocal_approx_fast(iva[:, hh * 4:(hh + 1) * 4],
                                                     sq[:])

                def norm_t(t):
                    tmp = hps_l[t]
                    nc.gpsimd.tensor_scalar(tmp[:], tmp[:], iva[:, t:t + 1],
                                            None, op0=Alu.mult)
                    nc.sync.dma_start_transpose(hidT[:, :, t * 128:(t + 1) * 128],
                                                tmp[:])

                def gelu_half(hh):
                    for ki in range(KM):
                        sl = hidT[:, ki, hh * 512:(hh + 1) * 512]
                        nc.scalar.activation(sl, sl, Act.Gelu,
                                             scale=gln_sb[:, ki:ki + 1],
                                             bias=bln_sb[:, ki:ki + 1])

                def fin():
                    if debug and c == 0:
                        nc.sync.dma_start(dbg["dhidT"].ap(), hidT[:])
                        nc.sync.dma_start(dbg["dcoef"].ap(), coef[:])

                out = []
                for hh in range(2):
                    for t in range(4 * hh, 4 * hh + 4):
                        out.append(lambda t=t: stats_t(t))
                    out.append(lambda hh=hh: half_iv(hh))
                    for t in range(4 * hh, 4 * hh + 4):
                        out.append(lambda t=t: norm_t(t))
                    out.append(lambda hh=hh: gelu_half(hh))
                return out + [fin]

            # ----- S3: mixed (fp16 matmul) -> mix8 + mixN (staged transpose) --
            def s3_tiles(c):
                hidT = st[c]["hidT"]
                mix8 = ws.tile([128, KH, CHUNK], f8, tag=f"m8{c}")
                mixN = ws.tile([128, NT, H], fp16, tag=f"mN{c}")
                st[c]["mix8"] = mix8
                st[c]["mixN"] = mixN

                def tile_m(m):
                    mps = psum_big()
                    for n in range(2):
                        for ki in range(KM):
                            nc.tensor.matmul(mps[:, n * 512:(n + 1) * 512],
                                             w2_sb[:, ki, m * 128:(m + 1) * 128],
                                             hidT[:, ki, n * 512:(n + 1) * 512],
                                             start=(ki == 0), stop=(ki == KM - 1))
                    mt = sm.tile([128, CHUNK], fp16, tag="mt", bufs=3)
                    nc.scalar.activation(mt[:], mps[:], Act.Identity,
                                         bias=b2_sb[:, m:m + 1])
                    nc.vector.tensor_scalar(mix8[:, m, :], mps[:],
                                            b2_sb[:, m:m + 1], S_M8,
                                            op0=Alu.add, op1=Alu.mult)
                    nc.sync.dma_start_transpose(mixN[:, :, m * 128:(m + 1) * 128],
                                                mt[:])

                def fin():
                    if debug and c == 0:
                        nc.sync.dma_start(dbg["dmix8"].ap(), mix8[:])
                        nc.sync.dma_start(dbg["dmixN"].ap(), mixN[:])

                return [lambda m=m: tile_m(m) for m in range(KH)] + [fin]

            # ----- S4: q/k/v projections (fp8 DoubleRow) -----
            def s4_tiles(c):
                mix8 = st[c]["mix8"]
                qT = ws.tile([128, KH, CHUNK], f8, tag=f"q8{c}")
                kT = ws.tile([128, KH, CHUNK], f8, tag=f"k8{c}")
                vN = ws.tile([128, NT, H], f8, tag=f"hvy{c}")
                st[c]["qT"] = qT
                st[c]["kT"] = kT
                st[c]["vN"] = vN

                def proj_m(dst, wsb, m, on_vec):
                    qps = psum_big()
                    for n in range(2):
                        for g in range(4):
                            nc.tensor.matmul(
                                qps[:, n * 512:(n + 1) * 512],
                                wsb[:, 2 * g:2 * g + 2, m * 128:(m + 1) * 128],
                                mix8[:, 2 * g:2 * g + 2, n * 512:(n + 1) * 512],
                                start=(g == 0), stop=(g == 3), perf_mode=DR)
                    sc = S_Q8 / (S_M8 * S_W8)
                    if on_vec:
                        nc.vector.tensor_scalar(dst[:, m, :], qps[:], sc, None,
                                                op0=Alu.mult)
                    else:
                        nc.scalar.activation(dst[:, m, :], qps[:], Act.Copy,
                                             scale=sc)

                def v_t(t):
                    vps = psum_big()
                    for n in range(2):
                        for g in range(4):
                            nc.tensor.matmul(
                                vps[:, n * 512:(n + 1) * 512],
                                mix8[:, 2 * g:2 * g + 2, t * 128:(t + 1) * 128],
                                wv_sb[:, 2 * g:2 * g + 2, n * 512:(n + 1) * 512],
                                start=(g == 0), stop=(g == 3), perf_mode=DR)
                    nc.vector.tensor_scalar(vN[:, t, :], vps[:],
                                            S_V8 / (S_M8 * S_W8), None,
                                            op0=Alu.mult)

                thunks = []
                for m in range(KH):
                    thunks.append(lambda m=m: proj_m(qT, wq_sb, m, False))
                for m in range(KH):
                    thunks.append(lambda m=m: proj_m(kT, wk_sb, m, c == 1))
                for t in range(NT):
                    thunks.append(lambda t=t: v_t(t))

                def fin():
                    if debug and c == 0:
                        nc.sync.dma_start(dbg["dqT"].ap(), qT[:])
                        nc.sync.dma_start(dbg["dkT"].ap(), kT[:])
                        nc.sync.dma_start(dbg["dvN"].ap(), vN[:])
                thunks.append(fin)
                return thunks

            # ----- S5: attention per head (scores -> exp -> PV+denom -> ocat) --
            def s5_scores(c, h, drain):
                qT, kT = st[c]["qT"], st[c]["kT"]
                if h == 0:
                    st[c]["ocat"] = ws.tile([128, NT, HD], fp16,
                                            tag=f"oc{c}", name="ocat")
                    st[c]["et"] = [None, None]
                    st[c]["et"][0] = ws.tile([128, KH, CHUNK], f8,
                                             tag=f"xet{c}", name="et0")
                    st[c]["et"][1] = ws.tile([128, KH, CHUNK], f8,
                                             tag=f"et1{c}", name="et1")
                    st[c]["otc"] = ws.tile([128, KH, CHUNK], f8,
                                           tag=f"m8{c}", name="otc")
                et = st[c]["et"][h % 2]
                exp_scale = float(HD ** -0.5) / (S_Q8 * S_Q8)

                for kt in range(NT):
                    stp = psum_st()
                    for qn in range(2):
                        nc.tensor.matmul(
                            stp[:, qn * 512:(qn + 1) * 512],
                            kT[:, 2 * h:2 * h + 2, kt * 128:(kt + 1) * 128],
                            qT[:, 2 * h:2 * h + 2, qn * 512:(qn + 1) * 512],
                            start=True, stop=True, perf_mode=DR)
                    nc.scalar.activation(et[:, kt, :], stp[:], Act.Exp,
                                         scale=exp_scale, bias=lns_sb[:])
                    drain(1)
                if debug and c == 0 and h == NUM_HEADS - 1:
                    nc.sync.dma_start(dbg["det"].ap(), et[:])

            def s5_pv(c, h, drain):
                vN = st[c]["vN"]
                ocat = st[c]["ocat"]
                otc = st[c]["otc"]
                et = st[c]["et"][h % 2]
                # denominator: d[q] = sum_k et * C_ONE, one column per qt
                dps = psum_st()
                for qt in range(NT):
                    for g in range(4):
                        nc.tensor.matmul(
                            dps[:, qt:qt + 1],
                            et[:, 2 * g:2 * g + 2, qt * 128:(qt + 1) * 128],
                            ones8[:], start=(qt == 0 and g == 0), stop=(g == 3),
                            perf_mode=DR, skip_group_check=True)
                rq = sm.tile([128, NT], f32, tag="rq", bufs=2)
                nc.vector.reciprocal_approx_fast(rq[:], dps[:, :NT])
                if debug and c == 0:
                    nc.sync.dma_start(dbg["ddn"].ap()[:, h, :], rq[:])
                drain(1)
                for qt in range(NT):
                    pvp = psum_st()[:, :HD]
                    for g in range(4):
                        nc.tensor.matmul(
                            pvp[:],
                            et[:, 2 * g:2 * g + 2, qt * 128:(qt + 1) * 128],
                            vN[:, 2 * g:2 * g + 2, h * HD:(h + 1) * HD],
                            start=(g == 0), stop=(g == 3), perf_mode=DR)
                    dst = ocat[:, qt, :]
                    nc.vector.tensor_scalar(dst, pvp[:], rq[:, qt:qt + 1],
                                            None, op0=Alu.mult)
                    if qt % 3 == 2:
                        drain(1)
                # transpose this head's output into the fp8 feature-major otc
                for qt in range(NT):
                    ot = sm.tile([128, 2, 128], fp16, tag="ott", bufs=2)
                    nc.sync.dma_start_transpose(ot[:], ocat[:, qt, :])
                    nc.gpsimd.tensor_copy(
                        otc[:, 2 * h:2 * h + 2, qt * 128:(qt + 1) * 128], ot[:])
                    if qt % 4 == 3:
                        drain(1)

            def s5_fin(c):
                if debug and c == 0:
                    nc.sync.dma_start(dbg["dotc"].ap(), st[c]["otc"][:])

            # ----- S6+S7: wo proj + residual + LN + output proj -------------
            # Two-pass LN: per-t stats are collected into mvall, then sqrt and
            # reciprocal run once batched (avoids Act Exp<->Sqrt table thrash
            # during the overlapped attention of the other chunk).
            def s67_tiles(c):
                otc8 = st[c]["otc"]  # [128, KH, CHUNK] fp8
                mixN = st[c]["mixN"]
                ych = ws.tile([128, NT, G], f32, tag=f"hvy{c}", name="ych")
                mvall = sm.tile([128, NT, 2], f32, tag="mvall", bufs=2,
                                name="mvall")
                iva = sm.tile([128, NT], f32, tag="iva", bufs=2, name="iva")

                def stats_t(t):
                    ops_ = psum_big()
                    for n in range(2):
                        for g in range(4):
                            nc.tensor.matmul(
                                ops_[:, n * 512:(n + 1) * 512],
                                otc8[:, 2 * g:2 * g + 2, t * 128:(t + 1) * 128],
                                wo_sb[:, 2 * g:2 * g + 2, n * 512:(n + 1) * 512],
                                start=(g == 0), stop=(g == 3), perf_mode=DR)
                    res = mixN[:, t, :]
                    nc.vector.scalar_tensor_tensor(res, ops_[:], INV_WO, res,
                                                   op0=Alu.mult, op1=Alu.add)
                    st6 = sm.tile([128, 2, 6], f32, tag="st6b")
                    for half in range(2):
                        nc.vector.bn_stats(st6[:, half, :],
                                           mixN[:, t, half * 512:(half + 1) * 512])
                    nc.vector.bn_aggr(mvall[:, t, :], st6[:])

                def batch_iv(hh):
                    sq = sm.tile([128, 4], f32, tag="sqb", name="sqb")
                    nc.scalar.activation(sq[:], mvall[:, hh * 4:(hh + 1) * 4, 1],
                                         Act.Sqrt, bias=eps_sb[:])
                    nc.vector.reciprocal_approx_fast(iva[:, hh * 4:(hh + 1) * 4],
                                                     sq[:])

                def norm_t(t):
                    res = mixN[:, t, :]
                    nc.gpsimd.tensor_scalar(res, res, mvall[:, t, 0:1],
                                            iva[:, t:t + 1],
                                            op0=Alu.subtract, op1=Alu.mult)
                    zst = sm.tile([128, KH, 128], fp16, tag="zst", bufs=2)
                    nc.sync.dma_start_transpose(zst[:], res)
                    yps = psum_big()[:, :G]
                    for i in range(KH):
                        nc.tensor.matmul(yps[:], zst[:, i, :],
                                         gw_sb[:, i, :],
                                         start=(i == 0), stop=(i == KH - 1))
                    nc.vector.tensor_tensor(ych[:, t, :], yps[:], bw_sb[:],
                                            op=Alu.add)
                    if debug and c == 0 and t == NT - 1:
                        nc.sync.dma_start(dbg["dres"].ap(), mixN[:])

                def fin():
                    for hh in range(2):
                        nc.sync.dma_start(
                            y.ap()[c, hh * 512:(hh + 1) * 512, :].rearrange(
                                "(t p) g -> p t g", p=128),
                            ych[:, hh * 4:(hh + 1) * 4, :])

                out = []
                for hh in range(2):
                    for t in range(4 * hh, 4 * hh + 4):
                        out.append(lambda t=t: stats_t(t))
                    out.append(lambda hh=hh: batch_iv(hh))
                    for t in range(4 * hh, 4 * hh + 4):
                        out.append(lambda t=t: norm_t(t))
                return out + [fin]

            # ---------------- schedule ----------------
            from collections import deque

            STAGE_MARKS.clear()

            def mark(label):
                n = int(nc.get_next_instruction_name().split("-")[1])
                STAGE_MARKS.append((label, n))

            def mk_drain(q):
                def drain(n):
                    for _ in range(min(n, len(q))):
                        q.popleft()()
                return drain

            a, b = 0, 1
            mark("s01a")
            s01_load(a)
            s01_load(b)
            load_big_weights()
            s01_mm(a)
            mark("s2a")
            qa = deque(s2_tiles(a))
            while qa:
                qa.popleft()()
            mark("s01b")
            s01_mm(b)
            # A.S3 with B.S2 interleaved
            mark("s3a+s2b")
            qb = deque(s2_tiles(b))
            for i, th in enumerate(s3_tiles(a)):
                th()
                if qb:
                    qb.popleft()()
            while qb:
                qb.popleft()()
            # A.S4 solid
            mark("s4a")
            for th in s4_tiles(a):
                th()
            # A.S5 with B.S3 + B.S4 as filler
            mark("s5a+s34b")
            qb = deque(s3_tiles(b) + s4_tiles(b))
            drain_b = mk_drain(qb)
            s5_scores(a, 0, drain_b)
            for h in range(1, NUM_HEADS):
                s5_scores(a, h, drain_b)
                s5_pv(a, h - 1, drain_b)
            s5_pv(a, NUM_HEADS - 1, drain_b)
            while qb:
                qb.popleft()()
            s5_fin(a)
            # B.S5 with A.S6+S7 as filler
            mark("s5b+s67a")
            qa = deque(s67_tiles(a))
            drain_a = mk_drain(qa)
            s5_scores(b, 0, drain_a)
            for h in range(1, NUM_HEADS):
                s5_scores(b, h, drain_a)
                s5_pv(b, h - 1, drain_a)
            s5_pv(b, NUM_HEADS - 1, drain_a)
            while qa:
                qa.popleft()()
            s5_fin(b)
            # B tail
            mark("s67b")
            for th in s67_tiles(b):
                th()
            mark("end")

    nc.compile()
    return nc


def _get_compiled():
    global _COMPILED
    if _COMPILED is None:
        _COMPILED = _build()
    return _COMPILED


def _prep_inputs(inputs):
    f32 = np.float32

    def a(name):
        return np.asarray(inputs[name], dtype=f32)

    x = a("x")
    mw = a("mother_wavelets")
    scales = a("scales")
    norm = np.sqrt(np.sum(mw ** 2, axis=2, keepdims=True))
    kern = (mw / np.maximum(norm, 1e-12)) * (1.0 / (1.0 + np.exp(-scales)))
    kern = kern[0, :, :, 0]                      # (W, H)
    kernT = np.ascontiguousarray(kern.T).astype(F16)

    w1a = np.concatenate([a("mix_w1"), a("mix_b1")[None, :]], axis=0).astype(F16)
    gln = np.ascontiguousarray(a("mix_ln_g").reshape(KM, 128).T).astype(f32)
    bln = np.ascontiguousarray(a("mix_ln_b").reshape(KM, 128).T).astype(f32)
    w2 = a("mix_w2").astype(F16)
    b2c = np.ascontiguousarray(a("mix_b2").reshape(KH, 128).T).astype(f32)
    gw = (a("out_ln_g")[:, None] * a("out_w")).astype(F16)
    bw_vec = a("out_ln_b") @ a("out_w") + a("out_b")
    bw = np.tile(bw_vec[None, :], (128, 1)).astype(f32)

    def to8(w, s):
        ws = w * s
        am = np.abs(ws).max()
        assert am < 224.0, f"fp8 overflow: {am}"
        return ws.astype(F8)

    wq = to8(a("wq"), S_W8)
    wk = to8(a("wk"), S_W8)
    wv = to8(a("wv"), S_W8)
    wo = to8(a("wo"), S_W8)

    shared = {
        "kernt": kernT, "w1a": w1a, "gln": gln, "bln": bln, "w2": w2,
        "b2c": b2c, "wq8": wq, "wk8": wk, "wv8": wv, "wo8": wo,
        "gw": gw, "bw": bw,
    }

    xc = x.reshape(N_CHUNKS, CHUNK, H)
    xt_all = np.ascontiguousarray(xc.transpose(0, 2, 1)).astype(F16)
    in_maps = []
    for core in range(N_CORES):
        m = dict(shared)
        m["xt"] = np.ascontiguousarray(xt_all[core * CPC:(core + 1) * CPC])
        in_maps.append(m)
    return in_maps


def kernel(**inputs) -> np.ndarray:
    from concourse.bass_utils import run_bass_kernel_spmd

    nc = _get_compiled()
    in_maps = _prep_inputs(inputs)
    res = run_bass_kernel_spmd(nc, in_maps, core_ids=list(range(N_CORES)))
    out = np.concatenate([r["y"] for r in res.results], axis=0)  # (16, CHUNK, G)
    return out.reshape(B, S, G).astype(np.float32)


# revision 45
# speedup vs baseline: 1.4140x; 1.0161x over previous
"""Trainium2 Bass kernel for nn_EntropyLM (wavelet-coeff mixer + chunked MHA + output proj).

Strategy: data-parallel over the 16 independent (batch x chunk) blocks, 2 per
NeuronCore.  The numerically-critical path (wavelet coeffs, mixer, residual
stream, output projection) runs in fp16 on the PE (same speed as bf16, 8x the
mantissa); the error-tolerant bulk (q/k/v projections, attention scores, PV,
attention-out projection) runs in fp8 e4m3 with DoubleRow perf mode, which
contracts K=256 per instruction at 0.5 cycles/row -- 4x bf16 matmul
throughput in the HW cost model.

Per-tensor power-of-two scales keep fp8 operands in [~1, 200]; all scale
corrections are folded into PSUM-evacuation ops that are needed anyway.

Layouts per chunk (CHUNK=1024 tokens, H=1024 features):
  * "T" tensors are feature-major [feat_part, ktile, token]; "N" tensors are
    token-major [token_part, ttile, feat].
  * Attention-out (ocat, token-major fp8) is transposed for the wo matmul by
    viewing fp8 pairs as uint16 through the DMA xbar transpose; the row
    permutation this induces on the contraction index is compensated by
    pre-permuting wo's rows on the host (wo8p).
  * The softmax denominator comes from a 1-column DoubleRow matmul against a
    constant 0.125 vector (reusing the PV lhsT weights); normalization is a
    per-partition scale on the PV evacuation.

The two chunks per core are software-pipelined by emission order: chunk B's
PE-heavy projection tiles are drained as filler between chunk A's Act-bound
attention pieces so the PE never idles waiting on exp().
"""

import numpy as np
import ml_dtypes

B, S, H, G, W = 4, 4096, 1024, 256, 8
CHUNK = 1024
NUM_HEADS = 4
HD = H // NUM_HEADS          # 256 per-head dim
HM = H // 2                  # 512 mixer hidden
N_CHUNKS = B * (S // CHUNK)  # 16 independent chunks
N_CORES = 8
CPC = N_CHUNKS // N_CORES    # 2 chunks per core
NT = CHUNK // 128            # 8 token tiles
KH = H // 128                # 8 feature tiles (H)
KM = HM // 128               # 4 feature tiles (HM)
EPS = 1e-5
BF16 = ml_dtypes.bfloat16
F8 = ml_dtypes.float8_e4m3
F16 = np.float16

# fp8 scales (powers of two; folded into evacuation ops)
S_W8 = 1024.0    # wq/wk/wv/wo weight scale
S_M8 = 64.0      # mix8 activation scale
S_Q8 = 128.0     # q/k fp8 scale
S_V8 = 128.0     # v fp8 scale
S_ET = 16.0      # exp(score) scale
C_ONE = 0.5      # denominator ones value -> ocat = (S_V8/C_ONE) * o = 256*o
S_O8 = S_V8 / C_ONE              # 1024
INV_WO = 1.0 / (S_O8 * S_W8)     # 2^-20

_COMPILED = None
STAGE_MARKS = []


def _build(debug=False):
    import concourse.bass as bass  # noqa: F401
    import concourse.tile as tile
    from concourse import bacc, mybir

    f8 = mybir.dt.float8e4
    u16 = mybir.dt.uint16
    fp16 = mybir.dt.float16
    f32 = mybir.dt.float32
    Alu = mybir.AluOpType
    Act = mybir.ActivationFunctionType
    DR = mybir.MatmulPerfMode.DoubleRow

    nc = bacc.Bacc("TRN2", target_bir_lowering=False, debug=False,
                   enable_asserts=True, num_devices=N_CORES)

    # ---- DRAM tensors (per-core views; same NEFF on all 8 cores) ----
    xt = nc.dram_tensor("xt", [CPC, H, CHUNK], fp16, kind="ExternalInput")
    kernT = nc.dram_tensor("kernt", [H, W], fp16, kind="ExternalInput")
    w1a = nc.dram_tensor("w1a", [W + 1, HM], fp16, kind="ExternalInput")
    gln = nc.dram_tensor("gln", [128, KM], f32, kind="ExternalInput")
    bln = nc.dram_tensor("bln", [128, KM], f32, kind="ExternalInput")
    w2 = nc.dram_tensor("w2", [HM, H], fp16, kind="ExternalInput")
    b2c = nc.dram_tensor("b2c", [128, KH], f32, kind="ExternalInput")
    wq8 = nc.dram_tensor("wq8", [H, H], f8, kind="ExternalInput")
    wk8 = nc.dram_tensor("wk8", [H, H], f8, kind="ExternalInput")
    wv8 = nc.dram_tensor("wv8", [H, H], f8, kind="ExternalInput")
    wo8 = nc.dram_tensor("wo8", [H, H], f8, kind="ExternalInput")
    gw = nc.dram_tensor("gw", [H, G], fp16, kind="ExternalInput")
    bw = nc.dram_tensor("bw", [128, G], f32, kind="ExternalInput")
    y = nc.dram_tensor("y", [CPC, CHUNK, G], f32, kind="ExternalOutput")
    dbg = {}
    if debug:
        for nm, shp, dt in [
            ("dcoef", [W + 1, CHUNK], fp16),
            ("dhidT", [128, KM, CHUNK], fp16),
            ("dmix8", [128, KH, CHUNK], f8),
            ("dmixN", [128, NT, H], fp16),
            ("dqT", [128, KH, CHUNK], f8),
            ("dkT", [128, KH, CHUNK], f8),
            ("dvN", [128, NT, H], f8),
            ("det", [128, KH, CHUNK], f8),
            ("ddn", [128, NUM_HEADS, NT], f32),
            ("dotc", [128, KH, CHUNK], f8),
            ("dres", [128, NT, H], fp16),
        ]:
            dbg[nm] = nc.dram_tensor(nm, shp, dt, kind="ExternalOutput")

    with tile.TileContext(nc) as tc:
        with (
            tc.tile_pool(name="wp", bufs=1) as wp,
            tc.tile_pool(name="ws", bufs=1) as ws,
            tc.tile_pool(name="sm", bufs=2) as sm,
            tc.tile_pool(name="ps", bufs=1, space="PSUM") as ps,
        ):
            # ---------- persistent weights ----------
            kt_sb = wp.tile([128, KH, W], fp16, tag="ktw")
            nc.sync.dma_start(kt_sb[:], kernT.ap().rearrange("(i p) w -> p i w", p=128))
            w1a_sb = wp.tile([W + 1, HM], fp16, tag="w1a")
            nc.sync.dma_start(w1a_sb[:], w1a.ap())
            gln_sb = wp.tile([128, KM], f32, tag="gln")
            nc.sync.dma_start(gln_sb[:], gln.ap())
            bln_sb = wp.tile([128, KM], f32, tag="bln")
            nc.sync.dma_start(bln_sb[:], bln.ap())
            b2_sb = wp.tile([128, KH], f32, tag="b2")
            nc.sync.dma_start(b2_sb[:], b2c.ap())
            w2_sb = wp.tile([128, KM, H], fp16, tag="w2s")
            wq_sb = wp.tile([128, KH, H], f8, tag="wq")
            wk_sb = wp.tile([128, KH, H], f8, tag="wk")
            wv_sb = wp.tile([128, KH, H], f8, tag="wv")
            wo_sb = wp.tile([128, KH, H], f8, tag="wo")
            gw_sb = wp.tile([128, KH, G], fp16, tag="gw")
            bw_sb = wp.tile([128, G], f32, tag="bw")

            def load_big_weights():
                # emitted after the x-stream DMAs so they don't delay S1
                nc.sync.dma_start(w2_sb[:],
                                  w2.ap().rearrange("(i p) m -> p i m", p=128))
                nc.scalar.dma_start(wq_sb[:],
                                    wq8.ap().rearrange("(i p) m -> p i m", p=128))
                nc.sync.dma_start(wk_sb[:],
                                  wk8.ap().rearrange("(i p) m -> p i m", p=128))
                nc.scalar.dma_start(wv_sb[:],
                                    wv8.ap().rearrange("(i p) m -> p i m", p=128))
                nc.sync.dma_start(gw_sb[:],
                                  gw.ap().rearrange("(i p) g -> p i g", p=128))
                nc.scalar.dma_start(bw_sb[:], bw.ap())
                nc.scalar.dma_start(wo_sb[:],
                                    wo8.ap().rearrange("(i p) m -> p i m", p=128))
            ones8 = wp.tile([128, 2, 1], f8, tag="ones")
            nc.vector.memset(ones8[:], C_ONE)
            eps_sb = wp.tile([128, 1], f32, tag="eps")
            nc.vector.memset(eps_sb[:], EPS)
            lns_sb = wp.tile([128, 1], f32, tag="lns")
            nc.vector.memset(lns_sb[:], float(np.log(S_ET)))

            # ---------- per-chunk state ----------
            st = [dict() for _ in range(CPC)]

            def psum_big(n=1024):
                return ps.tile([128, n], f32, tag="big", bufs=2, name="pbig")

            def psum_st():
                return ps.tile([128, 1024], f32, tag="st", bufs=2, name="pst")

            # ----- S0+S1: stream x (both queues), wavelet coeffs -----
            def s01_load(c):
                xf = ws.tile([128, KH, CHUNK], fp16, tag=f"xet{c}", name="xf")
                for ki in range(KH):
                    eng = nc.sync if ki % 2 == 0 else nc.scalar
                    eng.dma_start(xf[:, ki, :],
                                  xt.ap()[c, ki * 128:(ki + 1) * 128, :])
                st[c]["xs"] = xf

            def s01_mm(c):
                coef = ws.tile([W + 1, CHUNK], fp16, tag=f"coef{c}")
                nc.gpsimd.memset(coef[:, :], 1.0)
                cps = [psum_big(), psum_big()]
                xf = st[c]["xs"]
                for ki in range(KH):
                    for n in range(2):
                        nc.tensor.matmul(
                            cps[n][:W, :512], kt_sb[:, ki, :],
                            xf[:, ki, n * 512:(n + 1) * 512],
                            start=(ki == 0), stop=(ki == KH - 1))
                for n in range(2):
                    nc.scalar.copy(coef[:W, n * 512:(n + 1) * 512], cps[n][:W, :512])
                st[c]["coef"] = coef

            # ----- S2: mixer hidden + LN + gelu -> hidT (two-pass LN) -------
            def s2_tiles(c):
                coef = st[c]["coef"]
                hidT = ws.tile([128, KM, CHUNK], fp16, tag=f"hvy{c}")
                st[c]["hidT"] = hidT
                mva = sm.tile([128, NT, 2], f32, tag="mva2", bufs=2, name="mva")
                iva = sm.tile([128, NT], f32, tag="iva2", bufs=2, name="iva")
                hps_l = [None] * NT

                def stats_t(t):
                    hps = psum_big(512)
                    hps_l[t] = hps
                    nc.tensor.matmul(hps[:, :512], coef[:, t * 128:(t + 1) * 128],
                                     w1a_sb[:], start=True, stop=True)
                    st6 = sm.tile([128, 6], f32, tag="st6")
                    nc.vector.bn_stats(st6[:], hps[:, :512])
                    nc.vector.bn_aggr(mva[:, t, :], st6[:])
                    tmp = sm.tile([128, 512], fp16, tag="ntmp", bufs=4)
                    nc.vector.tensor_scalar(tmp[:], hps[:, :512],
                                            mva[:, t, 0:1], None,
                                            op0=Alu.subtract)
                    hps_l[t] = tmp

                def half_iv(hh):
                    sq = sm.tile([128, 4], f32, tag="sq2", name="sq2")
                    nc.scalar.activation(sq[:], mva[:, hh * 4:(hh + 1) * 4, 1],
                                         Act.Sqrt, bias=eps_sb[:])
                    nc.vector.reciprocal_approx_fast(iva[:, hh * 4:(hh + 1) * 4],
                                                     sq[:])

                def norm_t(t):
                    tmp = hps_l[t]
                    nc.gpsimd.tensor_scalar(tmp[:], tmp[:], iva[:, t:t + 1],
                                            None, op0=Alu.mult)
                    nc.sync.dma_start_transpose(hidT[:, :, t * 128:(t + 1) * 128],
                                                tmp[:])

                def gelu_half(hh):
                    for ki in range(KM):
                        sl = hidT[:, ki, hh * 512:(hh + 1) * 512]
                        nc.scalar.activation(sl, sl, Act.Gelu,
                                             scale=gln_sb[:, ki:ki + 1],
                                             bias=bln_sb[:, ki:ki + 1])

                def fin():
                    if debug and c == 0:
                        nc.sync.dma_start(dbg["dhidT"].ap(), hidT[:])
                        nc.sync.dma_start(dbg["dcoef"].ap(), coef[:])

                out = []
                for hh in range(2):
                    for t in range(4 * hh, 4 * hh + 4):
                        out.append(lambda t=t: stats_t(t))
                    out.append(lambda hh=hh: half_iv(hh))
                    for t in range(4 * hh, 4 * hh + 4):
                        out.append(lambda t=t: norm_t(t))
                    out.append(lambda hh=hh: gelu_half(hh))
                return out + [fin]

            # ----- S3: mixed (fp16 matmul) -> mix8 + mixN (staged transpose) --
            def s3_tiles(c):
                hidT = st[c]["hidT"]
                mix8 = ws.tile([128, KH, CHUNK], f8, tag=f"m8{c}")
                mixN = ws.tile([128, NT, H], fp16, tag=f"mN{c}")
                st[c]["mix8"] = mix8
                st[c]["mixN"] = mixN

                def tile_m(m):
                    mps = psum_big()
                    for n in range(2):
                        for ki in range(KM):
                            nc.tensor.matmul(mps[:, n * 512:(n + 1) * 512],
                                             w2_sb[:, ki, m * 128:(m + 1) * 128],
                                             hidT[:, ki, n * 512:(n + 1) * 512],
                                             start=(ki == 0), stop=(ki == KM - 1))
                    mt = sm.tile([128, CHUNK], fp16, tag="mt", bufs=3)
                    nc.scalar.activation(mt[:], mps[:], Act.Identity,
                                         bias=b2_sb[:, m:m + 1])
                    nc.vector.tensor_scalar(mix8[:, m, :], mps[:],
                                            b2_sb[:, m:m + 1], S_M8,
                                            op0=Alu.add, op1=Alu.mult)
                    nc.sync.dma_start_transpose(mixN[:, :, m * 128:(m + 1) * 128],
                                                mt[:])

                def fin():
                    if debug and c == 0:
                        nc.sync.dma_start(dbg["dmix8"].ap(), mix8[:])
                        nc.sync.dma_start(dbg["dmixN"].ap(), mixN[:])

                return [lambda m=m: tile_m(m) for m in range(KH)] + [fin]

            # ----- S4: q/k/v projections (fp8 DoubleRow) -----
            def s4_tiles(c):
                mix8 = st[c]["mix8"]
                qT = ws.tile([128, KH, CHUNK], f8, tag=f"q8{c}")
                kT = ws.tile([128, KH, CHUNK], f8, tag=f"k8{c}")
                vN = ws.tile([128, NT, H], f8, tag=f"hvy{c}")
                st[c]["qT"] = qT
                st[c]["kT"] = kT
                st[c]["vN"] = vN

                def proj_m(dst, wsb, m, on_vec):
                    qps = psum_big()
                    for n in range(2):
                        for g in range(4):
                            nc.tensor.matmul(
                                qps[:, n * 512:(n + 1) * 512],
                                wsb[:, 2 * g:2 * g + 2, m * 128:(m + 1) * 128],
                                mix8[:, 2 * g:2 * g + 2, n * 512:(n + 1) * 512],
                                start=(g == 0), stop=(g == 3), perf_mode=DR)
                    sc = S_Q8 / (S_M8 * S_W8)
                    if on_vec:
                        nc.vector.tensor_scalar(dst[:, m, :], qps[:], sc, None,
                                                op0=Alu.mult)
                    else:
                        nc.scalar.activation(dst[:, m, :], qps[:], Act.Copy,
                                             scale=sc)

                def v_t(t):
                    vps = psum_big()
                    for n in range(2):
                        for g in range(4):
                            nc.tensor.matmul(
                                vps[:, n * 512:(n + 1) * 512],
                                mix8[:, 2 * g:2 * g + 2, t * 128:(t + 1) * 128],
                                wv_sb[:, 2 * g:2 * g + 2, n * 512:(n + 1) * 512],
                                start=(g == 0), stop=(g == 3), perf_mode=DR)
                    nc.vector.tensor_scalar(vN[:, t, :], vps[:],
                                            S_V8 / (S_M8 * S_W8), None,
                                            op0=Alu.mult)

                thunks = []
                for m in range(KH):
                    thunks.append(lambda m=m: proj_m(qT, wq_sb, m, False))
                for m in range(KH):
                    thunks.append(lambda m=m: proj_m(kT, wk_sb, m, c == 1))
                for t in range(NT):
                    thunks.append(lambda t=t: v_t(t))

                def fin():
                    if debug and c == 0:
                        nc.sync.dma_start(dbg["dqT"].ap(), qT[:])
                        nc.sync.dma_start(dbg["dkT"].ap(), kT[:])
                        nc.sync.dma_start(dbg["dvN"].ap(), vN[:])
                thunks.append(fin)
                return thunks

            # ----- S5: attention per head (scores -> exp -> PV+denom -> ocat) --
            def s5_head(c, h, drain):
                qT, kT, vN = st[c]["qT"], st[c]["kT"], st[c]["vN"]
                if h == 0:
                    st[c]["ocat"] = ws.tile([128, NT, HD], fp16,
                                            tag=f"oc{c}", name="ocat")
                    st[c]["et"] = ws.tile([128, KH, CHUNK], f8,
                                          tag=f"xet{c}", name="et")
                    st[c]["otc"] = ws.tile([128, KH, CHUNK], f8,
                                           tag=f"m8{c}", name="otc")
                ocat = st[c]["ocat"]
                otc = st[c]["otc"]
                et = st[c]["et"]
                exp_scale = float(HD ** -0.5) / (S_Q8 * S_Q8)

                for kj in range(NT // 2):
                    stps = []
                    for kt in (2 * kj, 2 * kj + 1):
                        stp = psum_st()
                        stps.append(stp)
                        for qn in range(2):
                            nc.tensor.matmul(
                                stp[:, qn * 512:(qn + 1) * 512],
                                kT[:, 2 * h:2 * h + 2, kt * 128:(kt + 1) * 128],
                                qT[:, 2 * h:2 * h + 2, qn * 512:(qn + 1) * 512],
                                start=True, stop=True, perf_mode=DR)
                    for i, kt in enumerate((2 * kj, 2 * kj + 1)):
                        nc.scalar.activation(et[:, kt, :], stps[i][:], Act.Exp,
                                             scale=exp_scale, bias=lns_sb[:])
                    drain(1)
                if debug and c == 0 and h == NUM_HEADS - 1:
                    nc.sync.dma_start(dbg["det"].ap(), et[:])
                # PV with the denominator riding in column HD of the same
                # psum bank (same lhsT -> PE weight-load reuse); this removes
                # the separate denominator pass between exp and PV
                for qt in range(NT):
                    pvs = psum_st()
                    pvp = pvs[:, :HD]
                    for g in range(4):
                        nc.tensor.matmul(
                            pvp[:],
                            et[:, 2 * g:2 * g + 2, qt * 128:(qt + 1) * 128],
                            vN[:, 2 * g:2 * g + 2, h * HD:(h + 1) * HD],
                            start=(g == 0), stop=(g == 3), perf_mode=DR)
                        nc.tensor.matmul(
                            pvs[:, HD:HD + 1],
                            et[:, 2 * g:2 * g + 2, qt * 128:(qt + 1) * 128],
                            ones8[:], start=False, stop=(g == 3),
                            perf_mode=DR, skip_group_check=True)
                    rq = sm.tile([128, 1], f32, tag="rq", bufs=3)
                    nc.vector.reciprocal_approx_fast(rq[:], pvs[:, HD:HD + 1])
                    if debug and c == 0:
                        nc.sync.dma_start(dbg["ddn"].ap()[:, h, qt:qt + 1],
                                          rq[:])
                    dst = ocat[:, qt, :]
                    nc.vector.tensor_scalar(dst, pvp[:], rq[:],
                                            None, op0=Alu.mult)
                    if qt % 2 == 1:
                        drain(1)
                # transpose this head's output into the fp8 feature-major otc
                for qt in range(NT):
                    ot = sm.tile([128, 2, 128], fp16, tag="ott", bufs=2)
                    nc.sync.dma_start_transpose(ot[:], ocat[:, qt, :])
                    nc.gpsimd.tensor_copy(
                        otc[:, 2 * h:2 * h + 2, qt * 128:(qt + 1) * 128], ot[:])
                    if qt % 4 == 3:
                        drain(1)

            def s5_fin(c):
                if debug and c == 0:
                    nc.sync.dma_start(dbg["dotc"].ap(), st[c]["otc"][:])

            # ----- S6+S7: wo proj + residual + LN + output proj -------------
            # Two-pass LN: per-t stats are collected into mvall, then sqrt and
            # reciprocal run once batched (avoids Act Exp<->Sqrt table thrash
            # during the overlapped attention of the other chunk).
            def s67_tiles(c):
                otc8 = st[c]["otc"]  # [128, KH, CHUNK] fp8
                mixN = st[c]["mixN"]
                ych = ws.tile([128, NT, G], f32, tag=f"hvy{c}", name="ych")
                mvall = sm.tile([128, NT, 2], f32, tag="mvall", bufs=2,
                                name="mvall")
                iva = sm.tile([128, NT], f32, tag="iva", bufs=2, name="iva")

                def stats_t(t):
                    ops_ = psum_big()
                    for n in range(2):
                        for g in range(4):
                            nc.tensor.matmul(
                                ops_[:, n * 512:(n + 1) * 512],
                                otc8[:, 2 * g:2 * g + 2, t * 128:(t + 1) * 128],
                                wo_sb[:, 2 * g:2 * g + 2, n * 512:(n + 1) * 512],
                                start=(g == 0), stop=(g == 3), perf_mode=DR)
                    res = mixN[:, t, :]
                    nc.vector.scalar_tensor_tensor(res, ops_[:], INV_WO, res,
                                                   op0=Alu.mult, op1=Alu.add)
                    st6 = sm.tile([128, 2, 6], f32, tag="st6b")
                    for half in range(2):
                        nc.vector.bn_stats(st6[:, half, :],
                                           mixN[:, t, half * 512:(half + 1) * 512])
                    nc.vector.bn_aggr(mvall[:, t, :], st6[:])

                def batch_iv(hh):
                    sq = sm.tile([128, 4], f32, tag="sqb", name="sqb")
                    nc.scalar.activation(sq[:], mvall[:, hh * 4:(hh + 1) * 4, 1],
                                         Act.Sqrt, bias=eps_sb[:])
                    nc.vector.reciprocal_approx_fast(iva[:, hh * 4:(hh + 1) * 4],
                                                     sq[:])

                def norm_t(t):
                    res = mixN[:, t, :]
                    nc.gpsimd.tensor_scalar(res, res, mvall[:, t, 0:1],
                                            iva[:, t:t + 1],
                                            op0=Alu.subtract, op1=Alu.mult)
                    zst = sm.tile([128, KH, 128], fp16, tag="zst", bufs=2)
                    nc.sync.dma_start_transpose(zst[:], res)
                    yps = psum_big()[:, :G]
                    for i in range(KH):
                        nc.tensor.matmul(yps[:], zst[:, i, :],
                                         gw_sb[:, i, :],
                                         start=(i == 0), stop=(i == KH - 1))
                    nc.vector.tensor_tensor(ych[:, t, :], yps[:], bw_sb[:],
                                            op=Alu.add)
                    if debug and c == 0 and t == NT - 1:
                        nc.sync.dma_start(dbg["dres"].ap(), mixN[:])

                def fin():
                    for hh in range(2):
                        nc.sync.dma_start(
                            y.ap()[c, hh * 512:(hh + 1) * 512, :].rearrange(
                                "(t p) g -> p t g", p=128),
                            ych[:, hh * 4:(hh + 1) * 4, :])

                out = []
                for hh in range(2):
                    for t in range(4 * hh, 4 * hh + 4):
                        out.append(lambda t=t: stats_t(t))
                    out.append(lambda hh=hh: batch_iv(hh))
                    for t in range(4 * hh, 4 * hh + 4):
                        out.append(lambda t=t: norm_t(t))
                return out + [fin]

            # ---------------- schedule ----------------
            from collections import deque

            STAGE_MARKS.clear()

            def mark(label):
                n = int(nc.get_next_instruction_name().split("-")[1])
                STAGE_MARKS.append((label, n))

            def mk_drain(q):
                def drain(n):
                    for _ in range(min(n, len(q))):
                        q.popleft()()
                return drain

            a, b = 0, 1
            mark("s01a")
            s01_load(a)
            s01_load(b)
            load_big_weights()
            s01_mm(a)
            mark("s2a")
            qa = deque(s2_tiles(a))
            while qa:
                qa.popleft()()
            mark("s01b")
            s01_mm(b)
            # A.S3 with B.S2 interleaved
            mark("s3a+s2b")
            qb = deque(s2_tiles(b))
            for i, th in enumerate(s3_tiles(a)):
                th()
                if qb:
                    qb.popleft()()
            while qb:
                qb.popleft()()
            # A.S4 solid
            mark("s4a")
            for th in s4_tiles(a):
                th()
            # A.S5 with B.S3 + B.S4 as filler
            mark("s5a+s34b")
            qb = deque(s3_tiles(b) + s4_tiles(b))
            drain_b = mk_drain(qb)
            for h in range(NUM_HEADS):
                s5_head(a, h, drain_b)
            while qb:
                qb.popleft()()
            s5_fin(a)
            # B.S5 with A.S6+S7 as filler
            mark("s5b+s67a")
            qa = deque(s67_tiles(a))
            drain_a = mk_drain(qa)
            for h in range(NUM_HEADS):
                s5_head(b, h, drain_a)
            while qa:
                qa.popleft()()
            s5_fin(b)
            # B tail
            mark("s67b")
            for th in s67_tiles(b):
                th()
            mark("end")

    nc.compile()
    return nc


def _get_compiled():
    global _COMPILED
    if _COMPILED is None:
        _COMPILED = _build()
    return _COMPILED


def _prep_inputs(inputs):
    f32 = np.float32

    def a(name):
        return np.asarray(inputs[name], dtype=f32)

    x = a("x")
    mw = a("mother_wavelets")
    scales = a("scales")
    norm = np.sqrt(np.sum(mw ** 2, axis=2, keepdims=True))
    kern = (mw / np.maximum(norm, 1e-12)) * (1.0 / (1.0 + np.exp(-scales)))
    kern = kern[0, :, :, 0]                      # (W, H)
    kernT = np.ascontiguousarray(kern.T).astype(F16)

    w1a = np.concatenate([a("mix_w1"), a("mix_b1")[None, :]], axis=0).astype(F16)
    gln = np.ascontiguousarray(a("mix_ln_g").reshape(KM, 128).T).astype(f32)
    bln = np.ascontiguousarray(a("mix_ln_b").reshape(KM, 128).T).astype(f32)
    w2 = a("mix_w2").astype(F16)
    b2c = np.ascontiguousarray(a("mix_b2").reshape(KH, 128).T).astype(f32)
    gw = (a("out_ln_g")[:, None] * a("out_w")).astype(F16)
    bw_vec = a("out_ln_b") @ a("out_w") + a("out_b")
    bw = np.tile(bw_vec[None, :], (128, 1)).astype(f32)

    def to8(w, s):
        ws = w * s
        am = np.abs(ws).max()
        assert am < 224.0, f"fp8 overflow: {am}"
        return ws.astype(F8)

    wq = to8(a("wq"), S_W8)
    wk = to8(a("wk"), S_W8)
    wv = to8(a("wv"), S_W8)
    wo = to8(a("wo"), S_W8)

    shared = {
        "kernt": kernT, "w1a": w1a, "gln": gln, "bln": bln, "w2": w2,
        "b2c": b2c, "wq8": wq, "wk8": wk, "wv8": wv, "wo8": wo,
        "gw": gw, "bw": bw,
    }

    xc = x.reshape(N_CHUNKS, CHUNK, H)
    xt_all = np.ascontiguousarray(xc.transpose(0, 2, 1)).astype(F16)
    in_maps = []
    for core in range(N_CORES):
        m = dict(shared)
        m["xt"] = np.ascontiguousarray(xt_all[core * CPC:(core + 1) * CPC])
        in_maps.append(m)
    return in_maps


def kernel(**inputs) -> np.ndarray:
    from concourse.bass_utils import run_bass_kernel_spmd

    nc = _get_compiled()
    in_maps = _prep_inputs(inputs)
    res = run_bass_kernel_spmd(nc, in_maps, core_ids=list(range(N_CORES)))
    out = np.concatenate([r["y"] for r in res.results], axis=0)  # (16, CHUNK, G)
    return out.reshape(B, S, G).astype(np.float32)


# revision 52
# speedup vs baseline: 1.4271x; 1.0093x over previous
"""Trainium2 Bass kernel for nn_EntropyLM (wavelet-coeff mixer + chunked MHA + output proj).

Strategy: data-parallel over the 16 independent (batch x chunk) blocks, 2 per
NeuronCore.  The numerically-critical path (wavelet coeffs, mixer, residual
stream, output projection) runs in fp16 on the PE (same speed as bf16, 8x the
mantissa); the error-tolerant bulk (q/k/v projections, attention scores, PV,
attention-out projection) runs in fp8 e4m3 with DoubleRow perf mode, which
contracts K=256 per instruction at 0.5 cycles/row -- 4x bf16 matmul
throughput in the HW cost model.

Per-tensor power-of-two scales keep fp8 operands in [~1, 200]; all scale
corrections are folded into PSUM-evacuation ops that are needed anyway.

Layouts per chunk (CHUNK=1024 tokens, H=1024 features):
  * "T" tensors are feature-major [feat_part, ktile, token]; "N" tensors are
    token-major [token_part, ttile, feat].
  * Attention-out (ocat, token-major fp8) is transposed for the wo matmul by
    viewing fp8 pairs as uint16 through the DMA xbar transpose; the row
    permutation this induces on the contraction index is compensated by
    pre-permuting wo's rows on the host (wo8p).
  * The softmax denominator comes from a 1-column DoubleRow matmul against a
    constant 0.125 vector (reusing the PV lhsT weights); normalization is a
    per-partition scale on the PV evacuation.

The two chunks per core are software-pipelined by emission order: chunk B's
PE-heavy projection tiles are drained as filler between chunk A's Act-bound
attention pieces so the PE never idles waiting on exp().
"""

import numpy as np
import ml_dtypes

B, S, H, G, W = 4, 4096, 1024, 256, 8
CHUNK = 1024
NUM_HEADS = 4
HD = H // NUM_HEADS          # 256 per-head dim
HM = H // 2                  # 512 mixer hidden
N_CHUNKS = B * (S // CHUNK)  # 16 independent chunks
N_CORES = 8
CPC = N_CHUNKS // N_CORES    # 2 chunks per core
NT = CHUNK // 128            # 8 token tiles
KH = H // 128                # 8 feature tiles (H)
KM = HM // 128               # 4 feature tiles (HM)
EPS = 1e-5
BF16 = ml_dtypes.bfloat16
F8 = ml_dtypes.float8_e4m3
F16 = np.float16

# fp8 scales (powers of two; folded into evacuation ops)
S_W8 = 1024.0    # wq/wk/wv/wo weight scale
S_M8 = 64.0      # mix8 activation scale
S_Q8 = 128.0     # q/k fp8 scale
S_V8 = 128.0     # v fp8 scale
S_ET = 16.0      # exp(score) scale
C_ONE = 0.5      # denominator ones value -> ocat = (S_V8/C_ONE) * o = 256*o
S_O8 = S_V8 / C_ONE              # 1024
INV_WO = 1.0 / (S_O8 * S_W8)     # 2^-20

_COMPILED = None
STAGE_MARKS = []


def _build(debug=False):
    import concourse.bass as bass  # noqa: F401
    import concourse.tile as tile
    from concourse import bacc, mybir

    f8 = mybir.dt.float8e4
    u16 = mybir.dt.uint16
    fp16 = mybir.dt.float16
    f32 = mybir.dt.float32
    Alu = mybir.AluOpType
    Act = mybir.ActivationFunctionType
    DR = mybir.MatmulPerfMode.DoubleRow

    nc = bacc.Bacc("TRN2", target_bir_lowering=False, debug=False,
                   enable_asserts=True, num_devices=N_CORES)

    # ---- DRAM tensors (per-core views; same NEFF on all 8 cores) ----
    xt = nc.dram_tensor("xt", [CPC, H, CHUNK], fp16, kind="ExternalInput")
    kernT = nc.dram_tensor("kernt", [H, W], fp16, kind="ExternalInput")
    w1a = nc.dram_tensor("w1a", [W + 1, HM], fp16, kind="ExternalInput")
    gln = nc.dram_tensor("gln", [128, KM], f32, kind="ExternalInput")
    bln = nc.dram_tensor("bln", [128, KM], f32, kind="ExternalInput")
    w2 = nc.dram_tensor("w2", [HM, H], fp16, kind="ExternalInput")
    b2c = nc.dram_tensor("b2c", [128, KH], f32, kind="ExternalInput")
    wq8 = nc.dram_tensor("wq8", [H, H], f8, kind="ExternalInput")
    wk8 = nc.dram_tensor("wk8", [H, H], f8, kind="ExternalInput")
    wv8 = nc.dram_tensor("wv8", [H, H], f8, kind="ExternalInput")
    wo8 = nc.dram_tensor("wo8", [H, H], f8, kind="ExternalInput")
    gw = nc.dram_tensor("gw", [H, G], fp16, kind="ExternalInput")
    bw = nc.dram_tensor("bw", [128, G], f32, kind="ExternalInput")
    y = nc.dram_tensor("y", [CPC, CHUNK, G], f32, kind="ExternalOutput")
    dbg = {}
    if debug:
        for nm, shp, dt in [
            ("dcoef", [W + 1, CHUNK], fp16),
            ("dhidT", [128, KM, CHUNK], fp16),
            ("dmix8", [128, KH, CHUNK], f8),
            ("dmixN", [128, NT, H], fp16),
            ("dqT", [128, KH, CHUNK], f8),
            ("dkT", [128, KH, CHUNK], f8),
            ("dvN", [128, NT, H], f8),
            ("det", [128, KH, CHUNK], f8),
            ("ddn", [128, NUM_HEADS, NT], f32),
            ("dotc", [128, KH, CHUNK], f8),
            ("dres", [128, NT, H], fp16),
        ]:
            dbg[nm] = nc.dram_tensor(nm, shp, dt, kind="ExternalOutput")

    with tile.TileContext(nc) as tc:
        with (
            tc.tile_pool(name="wp", bufs=1) as wp,
            tc.tile_pool(name="ws", bufs=1) as ws,
            tc.tile_pool(name="sm", bufs=2) as sm,
            tc.tile_pool(name="ps", bufs=1, space="PSUM") as ps,
        ):
            # ---------- persistent weights ----------
            kt_sb = wp.tile([128, KH, W], fp16, tag="ktw")
            nc.sync.dma_start(kt_sb[:], kernT.ap().rearrange("(i p) w -> p i w", p=128))
            w1a_sb = wp.tile([W + 1, HM], fp16, tag="w1a")
            nc.sync.dma_start(w1a_sb[:], w1a.ap())
            gln_sb = wp.tile([128, KM], f32, tag="gln")
            nc.sync.dma_start(gln_sb[:], gln.ap())
            bln_sb = wp.tile([128, KM], f32, tag="bln")
            nc.sync.dma_start(bln_sb[:], bln.ap())
            b2_sb = wp.tile([128, KH], f32, tag="b2")
            nc.sync.dma_start(b2_sb[:], b2c.ap())
            w2_sb = wp.tile([128, KM, H], fp16, tag="w2s")
            wq_sb = wp.tile([128, KH, H], f8, tag="wq")
            wk_sb = wp.tile([128, KH, H], f8, tag="wk")
            wv_sb = wp.tile([128, KH, H], f8, tag="wv")
            wo_sb = wp.tile([128, KH, H], f8, tag="wo")
            gw_sb = wp.tile([128, KH, G], fp16, tag="gw")
            bw_sb = wp.tile([128, G], f32, tag="bw")

            def load_big_weights():
                # emitted after the x-stream DMAs so they don't delay S1
                nc.sync.dma_start(w2_sb[:],
                                  w2.ap().rearrange("(i p) m -> p i m", p=128))
                nc.scalar.dma_start(wq_sb[:],
                                    wq8.ap().rearrange("(i p) m -> p i m", p=128))
                nc.sync.dma_start(wk_sb[:],
                                  wk8.ap().rearrange("(i p) m -> p i m", p=128))
                nc.scalar.dma_start(wv_sb[:],
                                    wv8.ap().rearrange("(i p) m -> p i m", p=128))
                nc.sync.dma_start(gw_sb[:],
                                  gw.ap().rearrange("(i p) g -> p i g", p=128))
                nc.scalar.dma_start(bw_sb[:], bw.ap())
                nc.scalar.dma_start(wo_sb[:],
                                    wo8.ap().rearrange("(i p) m -> p i m", p=128))
            ones8 = wp.tile([128, 2, 1], f8, tag="ones")
            nc.vector.memset(ones8[:], C_ONE)
            eps_sb = wp.tile([128, 1], f32, tag="eps")
            nc.vector.memset(eps_sb[:], EPS)
            lns_sb = wp.tile([128, 1], f32, tag="lns")
            nc.vector.memset(lns_sb[:], float(np.log(S_ET)))

            # ---------- per-chunk state ----------
            st = [dict() for _ in range(CPC)]

            def psum_big(n=1024):
                return ps.tile([128, n], f32, tag="big", bufs=2, name="pbig")

            def psum_st():
                return ps.tile([128, 1024], f32, tag="st", bufs=2, name="pst")

            # ----- S0+S1: stream x (both queues), wavelet coeffs -----
            def s01_load(c):
                xf = ws.tile([128, KH, CHUNK], fp16, tag=f"xet{c}", name="xf")
                for ki in range(KH):
                    eng = nc.sync if ki % 2 == 0 else nc.scalar
                    eng.dma_start(xf[:, ki, :],
                                  xt.ap()[c, ki * 128:(ki + 1) * 128, :])
                st[c]["xs"] = xf

            def s01_mm(c):
                coef = ws.tile([W + 1, CHUNK], fp16, tag=f"coef{c}")
                nc.gpsimd.memset(coef[:, :], 1.0)
                cps = [psum_big(), psum_big()]
                xf = st[c]["xs"]
                for ki in range(KH):
                    for n in range(2):
                        nc.tensor.matmul(
                            cps[n][:W, :512], kt_sb[:, ki, :],
                            xf[:, ki, n * 512:(n + 1) * 512],
                            start=(ki == 0), stop=(ki == KH - 1))
                for n in range(2):
                    nc.scalar.copy(coef[:W, n * 512:(n + 1) * 512], cps[n][:W, :512])
                st[c]["coef"] = coef

            # ----- S2: mixer hidden + LN + gelu -> hidT (two-pass LN) -------
            def s2_tiles(c):
                coef = st[c]["coef"]
                hidT = ws.tile([128, KM, CHUNK], fp16, tag=f"hvy{c}")
                st[c]["hidT"] = hidT
                mva = sm.tile([128, NT, 2], f32, tag="mva2", bufs=2, name="mva")
                iva = sm.tile([128, NT], f32, tag="iva2", bufs=2, name="iva")
                hps_l = [None] * NT

                def stats_t(t):
                    hps = psum_big(512)
                    hps_l[t] = hps
                    nc.tensor.matmul(hps[:, :512], coef[:, t * 128:(t + 1) * 128],
                                     w1a_sb[:], start=True, stop=True)
                    st6 = sm.tile([128, 6], f32, tag="st6")
                    nc.vector.bn_stats(st6[:], hps[:, :512])
                    nc.vector.bn_aggr(mva[:, t, :], st6[:])
                    tmp = sm.tile([128, 512], fp16, tag="ntmp", bufs=4)
                    nc.vector.tensor_scalar(tmp[:], hps[:, :512],
                                            mva[:, t, 0:1], None,
                                            op0=Alu.subtract)
                    hps_l[t] = tmp

                def half_iv(hh):
                    sq = sm.tile([128, 4], f32, tag="sq2", name="sq2")
                    nc.scalar.activation(sq[:], mva[:, hh * 4:(hh + 1) * 4, 1],
                                         Act.Sqrt, bias=eps_sb[:])
                    nc.vector.reciprocal_approx_fast(iva[:, hh * 4:(hh + 1) * 4],
                                                     sq[:])

                def norm_t(t):
                    tmp = hps_l[t]
                    nc.gpsimd.tensor_scalar(tmp[:], tmp[:], iva[:, t:t + 1],
                                            None, op0=Alu.mult)
                    nc.sync.dma_start_transpose(hidT[:, :, t * 128:(t + 1) * 128],
                                                tmp[:])

                def gelu_half(hh):
                    for ki in range(KM):
                        sl = hidT[:, ki, hh * 512:(hh + 1) * 512]
                        nc.scalar.activation(sl, sl, Act.Gelu,
                                             scale=gln_sb[:, ki:ki + 1],
                                             bias=bln_sb[:, ki:ki + 1])

                def fin():
                    if debug and c == 0:
                        nc.sync.dma_start(dbg["dhidT"].ap(), hidT[:])
                        nc.sync.dma_start(dbg["dcoef"].ap(), coef[:])

                out = []
                for hh in range(2):
                    for t in range(4 * hh, 4 * hh + 4):
                        out.append(lambda t=t: stats_t(t))
                    out.append(lambda hh=hh: half_iv(hh))
                    for t in range(4 * hh, 4 * hh + 4):
                        out.append(lambda t=t: norm_t(t))
                    out.append(lambda hh=hh: gelu_half(hh))
                return out + [fin]

            # ----- S3: mixed (fp16 matmul) -> mix8 + mixN (staged transpose) --
            def s3_tiles(c):
                hidT = st[c]["hidT"]
                mix8 = ws.tile([128, KH, CHUNK], f8, tag=f"m8{c}")
                mixN = ws.tile([128, NT, H], fp16, tag=f"mN{c}")
                st[c]["mix8"] = mix8
                st[c]["mixN"] = mixN

                def tile_m(m):
                    mps = psum_big()
                    for n in range(2):
                        for ki in range(KM):
                            nc.tensor.matmul(mps[:, n * 512:(n + 1) * 512],
                                             w2_sb[:, ki, m * 128:(m + 1) * 128],
                                             hidT[:, ki, n * 512:(n + 1) * 512],
                                             start=(ki == 0), stop=(ki == KM - 1))
                    mt = sm.tile([128, CHUNK], fp16, tag="mt", bufs=3)
                    nc.scalar.activation(mt[:], mps[:], Act.Identity,
                                         bias=b2_sb[:, m:m + 1])
                    nc.vector.tensor_scalar(mix8[:, m, :], mps[:],
                                            b2_sb[:, m:m + 1], S_M8,
                                            op0=Alu.add, op1=Alu.mult)
                    nc.sync.dma_start_transpose(mixN[:, :, m * 128:(m + 1) * 128],
                                                mt[:])

                def fin():
                    if debug and c == 0:
                        nc.sync.dma_start(dbg["dmix8"].ap(), mix8[:])
                        nc.sync.dma_start(dbg["dmixN"].ap(), mixN[:])

                return [lambda m=m: tile_m(m) for m in range(KH)] + [fin]

            # ----- S4: q/k/v projections (fp8 DoubleRow) -----
            def s4_tiles(c):
                mix8 = st[c]["mix8"]
                qT = ws.tile([128, KH, CHUNK], f8, tag=f"q8{c}")
                kT = ws.tile([128, KH, CHUNK], f8, tag=f"k8{c}")
                vN = ws.tile([128, NT, H], f8, tag=f"hvy{c}")
                st[c]["qT"] = qT
                st[c]["kT"] = kT
                st[c]["vN"] = vN

                def proj_m(dst, wsb, m, on_vec):
                    qps = psum_big()
                    for n in range(2):
                        for g in range(4):
                            nc.tensor.matmul(
                                qps[:, n * 512:(n + 1) * 512],
                                wsb[:, 2 * g:2 * g + 2, m * 128:(m + 1) * 128],
                                mix8[:, 2 * g:2 * g + 2, n * 512:(n + 1) * 512],
                                start=(g == 0), stop=(g == 3), perf_mode=DR)
                    sc = S_Q8 / (S_M8 * S_W8)
                    if on_vec:
                        nc.vector.tensor_scalar(dst[:, m, :], qps[:], sc, None,
                                                op0=Alu.mult)
                    else:
                        nc.scalar.activation(dst[:, m, :], qps[:], Act.Copy,
                                             scale=sc)

                def v_t(t):
                    vps = psum_big()
                    for n in range(2):
                        for g in range(4):
                            nc.tensor.matmul(
                                vps[:, n * 512:(n + 1) * 512],
                                mix8[:, 2 * g:2 * g + 2, t * 128:(t + 1) * 128],
                                wv_sb[:, 2 * g:2 * g + 2, n * 512:(n + 1) * 512],
                                start=(g == 0), stop=(g == 3), perf_mode=DR)
                    nc.vector.tensor_scalar(vN[:, t, :], vps[:],
                                            S_V8 / (S_M8 * S_W8), None,
                                            op0=Alu.mult)

                thunks = []
                for m in range(KH):
                    thunks.append(lambda m=m: proj_m(qT, wq_sb, m, False))
                for m in range(KH):
                    thunks.append(lambda m=m: proj_m(kT, wk_sb, m, c == 1))
                for t in range(NT):
                    thunks.append(lambda t=t: v_t(t))

                def fin():
                    if debug and c == 0:
                        nc.sync.dma_start(dbg["dqT"].ap(), qT[:])
                        nc.sync.dma_start(dbg["dkT"].ap(), kT[:])
                        nc.sync.dma_start(dbg["dvN"].ap(), vN[:])
                thunks.append(fin)
                return thunks

            # ----- S5: attention per head (scores -> exp -> PV+denom -> ocat) --
            def s5_head(c, h, drain):
                qT, kT, vN = st[c]["qT"], st[c]["kT"], st[c]["vN"]
                if h == 0:
                    st[c]["ocat"] = ws.tile([128, NT, HD], fp16,
                                            tag=f"oc{c}", name="ocat")
                    st[c]["et"] = ws.tile([128, KH, CHUNK], f8,
                                          tag=f"xet{c}", name="et")
                    st[c]["otc"] = ws.tile([128, KH, CHUNK], f8,
                                           tag=f"m8{c}", name="otc")
                ocat = st[c]["ocat"]
                otc = st[c]["otc"]
                et = st[c]["et"]
                exp_scale = float(HD ** -0.5) / (S_Q8 * S_Q8)

                for kt in range(NT):
                    stp = psum_st()
                    for qn in range(2):
                        nc.tensor.matmul(
                            stp[:, qn * 512:(qn + 1) * 512],
                            kT[:, 2 * h:2 * h + 2, kt * 128:(kt + 1) * 128],
                            qT[:, 2 * h:2 * h + 2, qn * 512:(qn + 1) * 512],
                            start=True, stop=True, perf_mode=DR)
                    nc.scalar.activation(et[:, kt, :], stp[:], Act.Exp,
                                         scale=exp_scale, bias=lns_sb[:])
                    if kt % 2 == 1:
                        drain(1)
                if debug and c == 0 and h == NUM_HEADS - 1:
                    nc.sync.dma_start(dbg["det"].ap(), et[:])
                # PV with the denominator riding in column HD of the same
                # psum bank (same lhsT -> PE weight-load reuse); this removes
                # the separate denominator pass between exp and PV
                for qt in range(NT):
                    pvs = psum_st()
                    pvp = pvs[:, :HD]
                    for g in range(4):
                        nc.tensor.matmul(
                            pvp[:],
                            et[:, 2 * g:2 * g + 2, qt * 128:(qt + 1) * 128],
                            vN[:, 2 * g:2 * g + 2, h * HD:(h + 1) * HD],
                            start=(g == 0), stop=(g == 3), perf_mode=DR)
                        nc.tensor.matmul(
                            pvs[:, HD:HD + 1],
                            et[:, 2 * g:2 * g + 2, qt * 128:(qt + 1) * 128],
                            ones8[:], start=False, stop=(g == 3),
                            perf_mode=DR, skip_group_check=True)
                    rq = sm.tile([128, 1], f32, tag="rq", bufs=3)
                    nc.vector.reciprocal_approx_fast(rq[:], pvs[:, HD:HD + 1])
                    if debug and c == 0:
                        nc.sync.dma_start(dbg["ddn"].ap()[:, h, qt:qt + 1],
                                          rq[:])
                    dst = ocat[:, qt, :]
                    nc.vector.tensor_scalar(dst, pvp[:], rq[:],
                                            None, op0=Alu.mult)
                    if qt % 2 == 1:
                        drain(1)
                # transpose this head's output into the fp8 feature-major otc
                for qt in range(NT):
                    ot = sm.tile([128, 2, 128], fp16, tag="ott", bufs=2)
                    nc.sync.dma_start_transpose(ot[:], ocat[:, qt, :])
                    nc.gpsimd.tensor_copy(
                        otc[:, 2 * h:2 * h + 2, qt * 128:(qt + 1) * 128], ot[:])
                    if qt % 4 == 3:
                        drain(1)

            def s5_fin(c):
                if debug and c == 0:
                    nc.sync.dma_start(dbg["dotc"].ap(), st[c]["otc"][:])

            # ----- S6+S7: wo proj + residual + LN + output proj -------------
            # Two-pass LN: per-t stats are collected into mvall, then sqrt and
            # reciprocal run once batched (avoids Act Exp<->Sqrt table thrash
            # during the overlapped attention of the other chunk).
            def s67_tiles(c):
                otc8 = st[c]["otc"]  # [128, KH, CHUNK] fp8
                mixN = st[c]["mixN"]
                ych = ws.tile([128, NT, G], f32, tag=f"hvy{c}", name="ych")
                mvall = sm.tile([128, NT, 2], f32, tag="mvall", bufs=2,
                                name="mvall")
                iva = sm.tile([128, NT], f32, tag="iva", bufs=2, name="iva")

                def stats_t(t):
                    ops_ = psum_big()
                    for n in range(2):
                        for g in range(4):
                            nc.tensor.matmul(
                                ops_[:, n * 512:(n + 1) * 512],
                                otc8[:, 2 * g:2 * g + 2, t * 128:(t + 1) * 128],
                                wo_sb[:, 2 * g:2 * g + 2, n * 512:(n + 1) * 512],
                                start=(g == 0), stop=(g == 3), perf_mode=DR)
                    res = mixN[:, t, :]
                    nc.vector.scalar_tensor_tensor(res, ops_[:], INV_WO, res,
                                                   op0=Alu.mult, op1=Alu.add)
                    st6 = sm.tile([128, 2, 6], f32, tag="st6b")
                    for half in range(2):
                        nc.vector.bn_stats(st6[:, half, :],
                                           mixN[:, t, half * 512:(half + 1) * 512])
                    nc.vector.bn_aggr(mvall[:, t, :], st6[:])

                def batch_iv(hh):
                    sq = sm.tile([128, 4], f32, tag="sqb", name="sqb")
                    nc.scalar.activation(sq[:], mvall[:, hh * 4:(hh + 1) * 4, 1],
                                         Act.Sqrt, bias=eps_sb[:])
                    nc.vector.reciprocal_approx_fast(iva[:, hh * 4:(hh + 1) * 4],
                                                     sq[:])

                def norm_t(t):
                    res = mixN[:, t, :]
                    nc.gpsimd.tensor_scalar(res, res, mvall[:, t, 0:1],
                                            iva[:, t:t + 1],
                                            op0=Alu.subtract, op1=Alu.mult)
                    zst = sm.tile([128, KH, 128], fp16, tag="zst", bufs=2)
                    nc.sync.dma_start_transpose(zst[:], res)
                    yps = psum_big()[:, :G]
                    for i in range(KH):
                        nc.tensor.matmul(yps[:], zst[:, i, :],
                                         gw_sb[:, i, :],
                                         start=(i == 0), stop=(i == KH - 1))
                    nc.vector.tensor_tensor(ych[:, t, :], yps[:], bw_sb[:],
                                            op=Alu.add)
                    if debug and c == 0 and t == NT - 1:
                        nc.sync.dma_start(dbg["dres"].ap(), mixN[:])

                def fin():
                    for hh in range(2):
                        nc.sync.dma_start(
                            y.ap()[c, hh * 512:(hh + 1) * 512, :].rearrange(
                                "(t p) g -> p t g", p=128),
                            ych[:, hh * 4:(hh + 1) * 4, :])

                out = []
                for hh in range(2):
                    for t in range(4 * hh, 4 * hh + 4):
                        out.append(lambda t=t: stats_t(t))
                    out.append(lambda hh=hh: batch_iv(hh))
                    for t in range(4 * hh, 4 * hh + 4):
                        out.append(lambda t=t: norm_t(t))
                return out + [fin]

            # ---------------- schedule ----------------
            from collections import deque

            STAGE_MARKS.clear()

            def mark(label):
                n = int(nc.get_next_instruction_name().split("-")[1])
                STAGE_MARKS.append((label, n))

            def mk_drain(q, keep=0):
                state = {"i": 0}

                def drain(n):
                    state["i"] += 1
                    if keep and state["i"] % keep == 0:
                        return
                    for _ in range(min(n, len(q))):
                        q.popleft()()
                return drain

            a, b = 0, 1
            mark("s01a")
            s01_load(a)
            s01_load(b)
            load_big_weights()
            s01_mm(a)
            mark("s2a")
            qa = deque(s2_tiles(a))
            while qa:
                qa.popleft()()
            mark("s01b")
            s01_mm(b)
            # A.S3 + A.S4 with B.S2 interleaved across the whole stream
            mark("s34a+s2b")
            qb = deque(s2_tiles(b))
            for th in s3_tiles(a) + s4_tiles(a):
                th()
                if qb:
                    qb.popleft()()
            while qb:
                qb.popleft()()
            # A.S5 with B.S3 + B.S4 as filler
            mark("s5a+s34b")
            qb = deque(s3_tiles(b) + s4_tiles(b))
            drain_b = mk_drain(qb)
            for h in range(NUM_HEADS):
                s5_head(a, h, drain_b)
            while qb:
                qb.popleft()()
            s5_fin(a)
            # B.S5 with A.S6+S7 as filler
            mark("s5b+s67a")
            qa = deque(s67_tiles(a))
            drain_a = mk_drain(qa)
            for h in range(NUM_HEADS):
                s5_head(b, h, drain_a)
            while qa:
                qa.popleft()()
            s5_fin(b)
            # B tail
            mark("s67b")
            for th in s67_tiles(b):
                th()
            mark("end")

    nc.compile()
    return nc


def _get_compiled():
    global _COMPILED
    if _COMPILED is None:
        _COMPILED = _build()
    return _COMPILED


def _prep_inputs(inputs):
    f32 = np.float32

    def a(name):
        return np.asarray(inputs[name], dtype=f32)

    x = a("x")
    mw = a("mother_wavelets")
    scales = a("scales")
    norm = np.sqrt(np.sum(mw ** 2, axis=2, keepdims=True))
    kern = (mw / np.maximum(norm, 1e-12)) * (1.0 / (1.0 + np.exp(-scales)))
    kern = kern[0, :, :, 0]                      # (W, H)
    kernT = np.ascontiguousarray(kern.T).astype(F16)

    w1a = np.concatenate([a("mix_w1"), a("mix_b1")[None, :]], axis=0).astype(F16)
    gln = np.ascontiguousarray(a("mix_ln_g").reshape(KM, 128).T).astype(f32)
    bln = np.ascontiguousarray(a("mix_ln_b").reshape(KM, 128).T).astype(f32)
    w2 = a("mix_w2").astype(F16)
    b2c = np.ascontiguousarray(a("mix_b2").reshape(KH, 128).T).astype(f32)
    gw = (a("out_ln_g")[:, None] * a("out_w")).astype(F16)
    bw_vec = a("out_ln_b") @ a("out_w") + a("out_b")
    bw = np.tile(bw_vec[None, :], (128, 1)).astype(f32)

    def to8(w, s):
        ws = w * s
        am = np.abs(ws).max()
        assert am < 224.0, f"fp8 overflow: {am}"
        return ws.astype(F8)

    wq = to8(a("wq"), S_W8)
    wk = to8(a("wk"), S_W8)
    wv = to8(a("wv"), S_W8)
    wo = to8(a("wo"), S_W8)

    shared = {
        "kernt": kernT, "w1a": w1a, "gln": gln, "bln": bln, "w2": w2,
        "b2c": b2c, "wq8": wq, "wk8": wk, "wv8": wv, "wo8": wo,
        "gw": gw, "bw": bw,
    }

    xc = x.reshape(N_CHUNKS, CHUNK, H)
    xt_all = np.ascontiguousarray(xc.transpose(0, 2, 1)).astype(F16)
    in_maps = []
    for core in range(N_CORES):
        m = dict(shared)
        m["xt"] = np.ascontiguousarray(xt_all[core * CPC:(core + 1) * CPC])
        in_maps.append(m)
    return in_maps


def kernel(**inputs) -> np.ndarray:
    from concourse.bass_utils import run_bass_kernel_spmd

    nc = _get_compiled()
    in_maps = _prep_inputs(inputs)
    res = run_bass_kernel_spmd(nc, in_maps, core_ids=list(range(N_CORES)))
    out = np.concatenate([r["y"] for r in res.results], axis=0)  # (16, CHUNK, G)
    return out.reshape(B, S, G).astype(np.float32)


# revision 60
# speedup vs baseline: 1.4606x; 1.0235x over previous
"""Trainium2 Bass kernel for nn_EntropyLM (wavelet-coeff mixer + chunked MHA + output proj).

Strategy: data-parallel over the 16 independent (batch x chunk) blocks, 2 per
NeuronCore.  The numerically-critical path (wavelet coeffs, mixer, residual
stream, output projection) runs in fp16 on the PE (same speed as bf16, 8x the
mantissa); the error-tolerant bulk (q/k/v projections, attention scores, PV,
attention-out projection) runs in fp8 e4m3 with DoubleRow perf mode, which
contracts K=256 per instruction at 0.5 cycles/row -- 4x bf16 matmul
throughput in the HW cost model.

Per-tensor power-of-two scales keep fp8 operands in [~1, 200]; all scale
corrections are folded into PSUM-evacuation ops that are needed anyway.

Layouts per chunk (CHUNK=1024 tokens, H=1024 features):
  * "T" tensors are feature-major [feat_part, ktile, token]; "N" tensors are
    token-major [token_part, ttile, feat].
  * Attention-out (ocat, token-major fp8) is transposed for the wo matmul by
    viewing fp8 pairs as uint16 through the DMA xbar transpose; the row
    permutation this induces on the contraction index is compensated by
    pre-permuting wo's rows on the host (wo8p).
  * The softmax denominator comes from a 1-column DoubleRow matmul against a
    constant 0.125 vector (reusing the PV lhsT weights); normalization is a
    per-partition scale on the PV evacuation.

The two chunks per core are software-pipelined by emission order: chunk B's
PE-heavy projection tiles are drained as filler between chunk A's Act-bound
attention pieces so the PE never idles waiting on exp().
"""

import numpy as np
import ml_dtypes

B, S, H, G, W = 4, 4096, 1024, 256, 8
CHUNK = 1024
NUM_HEADS = 4
HD = H // NUM_HEADS          # 256 per-head dim
HM = H // 2                  # 512 mixer hidden
N_CHUNKS = B * (S // CHUNK)  # 16 independent chunks
N_CORES = 8
CPC = N_CHUNKS // N_CORES    # 2 chunks per core
NT = CHUNK // 128            # 8 token tiles
KH = H // 128                # 8 feature tiles (H)
KM = HM // 128               # 4 feature tiles (HM)
EPS = 1e-5
BF16 = ml_dtypes.bfloat16
F8 = ml_dtypes.float8_e4m3
F16 = np.float16

# fp8 scales (powers of two; folded into evacuation ops)
S_W8 = 1024.0    # wq/wk/wv/wo weight scale
S_M8 = 64.0      # mix8 activation scale
S_Q8 = 128.0     # q/k fp8 scale
S_V8 = 128.0     # v fp8 scale
S_ET = 16.0      # exp(score) scale
C_ONE = 0.5      # denominator ones value -> ocat = (S_V8/C_ONE) * o = 256*o
S_O8 = S_V8 / C_ONE              # 1024
INV_WO = 1.0 / (S_O8 * S_W8)     # 2^-20

_COMPILED = None
STAGE_MARKS = []


def _build(debug=False):
    import concourse.bass as bass  # noqa: F401
    import concourse.tile as tile
    from concourse import bacc, mybir

    f8 = mybir.dt.float8e4
    u16 = mybir.dt.uint16
    fp16 = mybir.dt.float16
    f32 = mybir.dt.float32
    Alu = mybir.AluOpType
    Act = mybir.ActivationFunctionType
    DR = mybir.MatmulPerfMode.DoubleRow

    nc = bacc.Bacc("TRN2", target_bir_lowering=False, debug=False,
                   enable_asserts=True, num_devices=N_CORES)

    # ---- DRAM tensors (per-core views; same NEFF on all 8 cores) ----
    xt = nc.dram_tensor("xt", [CPC, H, CHUNK], fp16, kind="ExternalInput")
    kernT = nc.dram_tensor("kernt", [H, W], fp16, kind="ExternalInput")
    w1a = nc.dram_tensor("w1a", [W + 1, HM], fp16, kind="ExternalInput")
    smallw = nc.dram_tensor("smallw", [128, 2 * KM + KH + G], f32,
                            kind="ExternalInput")
    w2 = nc.dram_tensor("w2", [HM, H], fp16, kind="ExternalInput")
    wq8 = nc.dram_tensor("wq8", [H, H], f8, kind="ExternalInput")
    wk8 = nc.dram_tensor("wk8", [H, H], f8, kind="ExternalInput")
    wv8 = nc.dram_tensor("wv8", [H, H], f8, kind="ExternalInput")
    wo8 = nc.dram_tensor("wo8", [H, H], f8, kind="ExternalInput")
    gw = nc.dram_tensor("gw", [H, G], fp16, kind="ExternalInput")
    y = nc.dram_tensor("y", [CPC, CHUNK, G], f32, kind="ExternalOutput")
    dbg = {}
    if debug:
        for nm, shp, dt in [
            ("dcoef", [W + 1, CHUNK], fp16),
            ("dhidT", [128, KM, CHUNK], fp16),
            ("dmix8", [128, KH, CHUNK], f8),
            ("dmixN", [128, NT, H], fp16),
            ("dqT", [128, KH, CHUNK], f8),
            ("dkT", [128, KH, CHUNK], f8),
            ("dvN", [128, NT, H], f8),
            ("det", [128, KH, CHUNK], f8),
            ("ddn", [128, NUM_HEADS, NT], f32),
            ("dotc", [128, KH, CHUNK], f8),
            ("dres", [128, NT, H], fp16),
        ]:
            dbg[nm] = nc.dram_tensor(nm, shp, dt, kind="ExternalOutput")

    with tile.TileContext(nc) as tc:
        with (
            tc.tile_pool(name="wp", bufs=1) as wp,
            tc.tile_pool(name="ws", bufs=1) as ws,
            tc.tile_pool(name="sm", bufs=2) as sm,
            tc.tile_pool(name="ps", bufs=1, space="PSUM") as ps,
        ):
            # ---------- persistent weights ----------
            kt_sb = wp.tile([128, KH, W], fp16, tag="ktw")
            nc.sync.dma_start(kt_sb[:], kernT.ap().rearrange("(i p) w -> p i w", p=128))
            w1a_sb = wp.tile([W + 1, HM], fp16, tag="w1a")
            nc.sync.dma_start(w1a_sb[:], w1a.ap())
            smallw_sb = wp.tile([128, 2 * KM + KH + G], f32, tag="smallw")
            gln_sb = smallw_sb[:, 0:KM]
            bln_sb = smallw_sb[:, KM:2 * KM]
            b2_sb = smallw_sb[:, 2 * KM:2 * KM + KH]
            bw_sb = smallw_sb[:, 2 * KM + KH:]
            w2_sb = wp.tile([128, KM, H], fp16, tag="w2s")
            wq_sb = wp.tile([128, KH, H], f8, tag="wq")
            wk_sb = wp.tile([128, KH, H], f8, tag="wk")
            wv_sb = wp.tile([128, KH, H], f8, tag="wv")
            wo_sb = wp.tile([128, KH, H], f8, tag="wo")
            gw_sb = wp.tile([128, KH, G], fp16, tag="gw")

            def load_big_weights():
                # emitted after the x-stream DMAs so they don't delay S1
                nc.scalar.dma_start(smallw_sb[:], smallw.ap())
                nc.sync.dma_start(w2_sb[:],
                                  w2.ap().rearrange("(i p) m -> p i m", p=128))
                nc.scalar.dma_start(wq_sb[:],
                                    wq8.ap().rearrange("(i p) m -> p i m", p=128))
                nc.sync.dma_start(wk_sb[:],
                                  wk8.ap().rearrange("(i p) m -> p i m", p=128))
                nc.scalar.dma_start(wv_sb[:],
                                    wv8.ap().rearrange("(i p) m -> p i m", p=128))
                nc.sync.dma_start(gw_sb[:],
                                  gw.ap().rearrange("(i p) g -> p i g", p=128))
                nc.scalar.dma_start(wo_sb[:],
                                    wo8.ap().rearrange("(i p) m -> p i m", p=128))
            ones8 = wp.tile([128, 2, 1], f8, tag="ones")
            nc.vector.memset(ones8[:], C_ONE)
            eps_sb = wp.tile([128, 1], f32, tag="eps")
            nc.vector.memset(eps_sb[:], EPS)
            lns_sb = wp.tile([128, 1], f32, tag="lns")
            nc.vector.memset(lns_sb[:], float(np.log(S_ET)))

            # ---------- per-chunk state ----------
            st = [dict() for _ in range(CPC)]

            def psum_big(n=1024):
                return ps.tile([128, n], f32, tag="big", bufs=2, name="pbig")

            def psum_st():
                return ps.tile([128, 1024], f32, tag="st", bufs=2, name="pst")

            # ----- S0+S1: stream x (both queues), wavelet coeffs -----
            def s01_load(c):
                xf = ws.tile([128, KH, CHUNK], fp16, tag=f"xet{c}", name="xf")
                for j in range(4):
                    eng = nc.sync if j % 2 == 0 else nc.scalar
                    eng.dma_start(
                        xf[:, 2 * j:2 * j + 2, :],
                        xt.ap()[c, j * 256:(j + 1) * 256, :].rearrange(
                            "(i p) t -> p i t", p=128))
                st[c]["xs"] = xf

            def s01_mm(c):
                coef = ws.tile([W + 1, CHUNK], fp16, tag=f"coef{c}")
                nc.gpsimd.memset(coef[:, :], 1.0)
                cps = [psum_big(), psum_big()]
                xf = st[c]["xs"]
                for ki in range(KH):
                    for n in range(2):
                        nc.tensor.matmul(
                            cps[n][:W, :512], kt_sb[:, ki, :],
                            xf[:, ki, n * 512:(n + 1) * 512],
                            start=(ki == 0), stop=(ki == KH - 1))
                for n in range(2):
                    nc.scalar.copy(coef[:W, n * 512:(n + 1) * 512], cps[n][:W, :512])
                st[c]["coef"] = coef

            # ----- S2: mixer hidden + LN + gelu -> hidT (two-pass LN) -------
            def s2_tiles(c):
                coef = st[c]["coef"]
                hidT = ws.tile([128, KM, CHUNK], fp16, tag=f"hvy{c}")
                st[c]["hidT"] = hidT
                mva = sm.tile([128, NT, 2], f32, tag="mva2", bufs=2, name="mva")
                iva = sm.tile([128, NT], f32, tag="iva2", bufs=2, name="iva")
                hps_l = [None] * NT

                def stats_t(t):
                    hps = psum_big(512)
                    hps_l[t] = hps
                    nc.tensor.matmul(hps[:, :512], coef[:, t * 128:(t + 1) * 128],
                                     w1a_sb[:], start=True, stop=True)
                    st6 = sm.tile([128, 6], f32, tag="st6")
                    nc.vector.bn_stats(st6[:], hps[:, :512])
                    nc.vector.bn_aggr(mva[:, t, :], st6[:])
                    tmp = sm.tile([128, 512], fp16, tag="ntmp", bufs=4)
                    nc.vector.tensor_scalar(tmp[:], hps[:, :512],
                                            mva[:, t, 0:1], None,
                                            op0=Alu.subtract)
                    hps_l[t] = tmp

                def half_iv(hh):
                    sq = sm.tile([128, 4], f32, tag="sq2", name="sq2")
                    nc.scalar.activation(sq[:], mva[:, hh * 4:(hh + 1) * 4, 1],
                                         Act.Sqrt, bias=eps_sb[:])
                    nc.vector.reciprocal_approx_fast(iva[:, hh * 4:(hh + 1) * 4],
                                                     sq[:])

                def norm_t(t):
                    tmp = hps_l[t]
                    nc.gpsimd.tensor_scalar(tmp[:], tmp[:], iva[:, t:t + 1],
                                            None, op0=Alu.mult)
                    nc.sync.dma_start_transpose(hidT[:, :, t * 128:(t + 1) * 128],
                                                tmp[:])

                def gelu_half(hh):
                    for ki in range(KM):
                        sl = hidT[:, ki, hh * 512:(hh + 1) * 512]
                        nc.scalar.activation(sl, sl, Act.Gelu,
                                             scale=gln_sb[:, ki:ki + 1],
                                             bias=bln_sb[:, ki:ki + 1])

                def fin():
                    if debug and c == 0:
                        nc.sync.dma_start(dbg["dhidT"].ap(), hidT[:])
                        nc.sync.dma_start(dbg["dcoef"].ap(), coef[:])

                out = []
                for hh in range(2):
                    for t in range(4 * hh, 4 * hh + 4):
                        out.append(lambda t=t: stats_t(t))
                    out.append(lambda hh=hh: half_iv(hh))
                    for t in range(4 * hh, 4 * hh + 4):
                        out.append(lambda t=t: norm_t(t))
                    out.append(lambda hh=hh: gelu_half(hh))
                return out + [fin]

            # ----- S3: mixed (fp16 matmul) -> mix8 + mixN (staged transpose) --
            def s3_tiles(c):
                hidT = st[c]["hidT"]
                mix8 = ws.tile([128, KH, CHUNK], f8, tag=f"m8{c}")
                mixN = ws.tile([128, NT, H], fp16, tag=f"mN{c}")
                st[c]["mix8"] = mix8
                st[c]["mixN"] = mixN

                def tile_m(m):
                    mps = psum_big()
                    for n in range(2):
                        for ki in range(KM):
                            nc.tensor.matmul(mps[:, n * 512:(n + 1) * 512],
                                             w2_sb[:, ki, m * 128:(m + 1) * 128],
                                             hidT[:, ki, n * 512:(n + 1) * 512],
                                             start=(ki == 0), stop=(ki == KM - 1))
                    mt = sm.tile([128, CHUNK], fp16, tag="mt", bufs=3)
                    nc.scalar.activation(mt[:], mps[:], Act.Identity,
                                         bias=b2_sb[:, m:m + 1])
                    nc.vector.tensor_scalar(mix8[:, m, :], mps[:],
                                            b2_sb[:, m:m + 1], S_M8,
                                            op0=Alu.add, op1=Alu.mult)
                    nc.sync.dma_start_transpose(mixN[:, :, m * 128:(m + 1) * 128],
                                                mt[:])

                def fin():
                    if debug and c == 0:
                        nc.sync.dma_start(dbg["dmix8"].ap(), mix8[:])
                        nc.sync.dma_start(dbg["dmixN"].ap(), mixN[:])

                return [lambda m=m: tile_m(m) for m in range(KH)] + [fin]

            # ----- S4: q/k/v projections (fp8 DoubleRow) -----
            def s4_tiles(c):
                mix8 = st[c]["mix8"]
                qT = ws.tile([128, KH, CHUNK], f8, tag=f"q8{c}")
                kT = ws.tile([128, KH, CHUNK], f8, tag=f"k8{c}")
                vN = ws.tile([128, NT, H], f8, tag=f"hvy{c}")
                st[c]["qT"] = qT
                st[c]["kT"] = kT
                st[c]["vN"] = vN

                def proj_m(dst, wsb, m, on_vec):
                    qps = psum_big()
                    for n in range(2):
                        for g in range(4):
                            nc.tensor.matmul(
                                qps[:, n * 512:(n + 1) * 512],
                                wsb[:, 2 * g:2 * g + 2, m * 128:(m + 1) * 128],
                                mix8[:, 2 * g:2 * g + 2, n * 512:(n + 1) * 512],
                                start=(g == 0), stop=(g == 3), perf_mode=DR)
                    sc = S_Q8 / (S_M8 * S_W8)
                    if on_vec:
                        nc.vector.tensor_scalar(dst[:, m, :], qps[:], sc, None,
                                                op0=Alu.mult)
                    else:
                        nc.scalar.activation(dst[:, m, :], qps[:], Act.Copy,
                                             scale=sc)

                def v_t(t):
                    vps = psum_big()
                    for n in range(2):
                        for g in range(4):
                            nc.tensor.matmul(
                                vps[:, n * 512:(n + 1) * 512],
                                mix8[:, 2 * g:2 * g + 2, t * 128:(t + 1) * 128],
                                wv_sb[:, 2 * g:2 * g + 2, n * 512:(n + 1) * 512],
                                start=(g == 0), stop=(g == 3), perf_mode=DR)
                    nc.vector.tensor_scalar(vN[:, t, :], vps[:],
                                            S_V8 / (S_M8 * S_W8), None,
                                            op0=Alu.mult)

                thunks = []
                for m in range(KH):
                    thunks.append(lambda m=m: proj_m(qT, wq_sb, m, False))
                for m in range(KH):
                    thunks.append(lambda m=m: proj_m(kT, wk_sb, m, c == 1))
                for t in range(NT):
                    thunks.append(lambda t=t: v_t(t))

                def fin():
                    if debug and c == 0:
                        nc.sync.dma_start(dbg["dqT"].ap(), qT[:])
                        nc.sync.dma_start(dbg["dkT"].ap(), kT[:])
                        nc.sync.dma_start(dbg["dvN"].ap(), vN[:])
                thunks.append(fin)
                return thunks

            # ----- S5: attention per head (scores -> exp -> PV+denom -> ocat) --
            def s5_head(c, h, drain):
                qT, kT, vN = st[c]["qT"], st[c]["kT"], st[c]["vN"]
                if h == 0:
                    st[c]["ocat"] = ws.tile([128, NT, HD], fp16,
                                            tag=f"oc{c}", name="ocat")
                    st[c]["et"] = ws.tile([128, KH, CHUNK], f8,
                                          tag=f"xet{c}", name="et")
                    st[c]["otc"] = ws.tile([128, KH, CHUNK], f8,
                                           tag=f"m8{c}", name="otc")
                ocat = st[c]["ocat"]
                otc = st[c]["otc"]
                et = st[c]["et"]
                exp_scale = float(HD ** -0.5) / (S_Q8 * S_Q8)

                for kt in range(NT):
                    stp = psum_st()
                    for qn in range(2):
                        nc.tensor.matmul(
                            stp[:, qn * 512:(qn + 1) * 512],
                            kT[:, 2 * h:2 * h + 2, kt * 128:(kt + 1) * 128],
                            qT[:, 2 * h:2 * h + 2, qn * 512:(qn + 1) * 512],
                            start=True, stop=True, perf_mode=DR)
                    nc.scalar.activation(et[:, kt, :], stp[:], Act.Exp,
                                         scale=exp_scale, bias=lns_sb[:])
                    if kt % 2 == 1:
                        drain(1)
                if debug and c == 0 and h == NUM_HEADS - 1:
                    nc.sync.dma_start(dbg["det"].ap(), et[:])
                # PV with the denominator riding in column HD of the same
                # psum bank (same lhsT -> PE weight-load reuse); this removes
                # the separate denominator pass between exp and PV
                for qt in range(NT):
                    pvs = psum_st()
                    pvp = pvs[:, :HD]
                    for g in range(4):
                        nc.tensor.matmul(
                            pvp[:],
                            et[:, 2 * g:2 * g + 2, qt * 128:(qt + 1) * 128],
                            vN[:, 2 * g:2 * g + 2, h * HD:(h + 1) * HD],
                            start=(g == 0), stop=(g == 3), perf_mode=DR)
                        nc.tensor.matmul(
                            pvs[:, HD:HD + 1],
                            et[:, 2 * g:2 * g + 2, qt * 128:(qt + 1) * 128],
                            ones8[:], start=False, stop=(g == 3),
                            perf_mode=DR, skip_group_check=True)
                    rq = sm.tile([128, 1], f32, tag="rq", bufs=3)
                    nc.vector.reciprocal_approx_fast(rq[:], pvs[:, HD:HD + 1])
                    if debug and c == 0:
                        nc.sync.dma_start(dbg["ddn"].ap()[:, h, qt:qt + 1],
                                          rq[:])
                    dst = ocat[:, qt, :]
                    nc.vector.tensor_scalar(dst, pvp[:], rq[:],
                                            None, op0=Alu.mult)
                    if qt % 2 == 1:
                        drain(1)
                # transpose this head's output into the fp8 feature-major otc
                for qt in range(NT):
                    ot = sm.tile([128, 2, 128], fp16, tag="ott", bufs=2)
                    nc.sync.dma_start_transpose(ot[:], ocat[:, qt, :])
                    nc.gpsimd.tensor_copy(
                        otc[:, 2 * h:2 * h + 2, qt * 128:(qt + 1) * 128], ot[:])
                    if qt % 4 == 3:
                        drain(1)

            def s5_fin(c):
                if debug and c == 0:
                    nc.sync.dma_start(dbg["dotc"].ap(), st[c]["otc"][:])

            # ----- S6+S7: wo proj + residual + LN + output proj -------------
            # Two-pass LN: per-t stats are collected into mvall, then sqrt and
            # reciprocal run once batched (avoids Act Exp<->Sqrt table thrash
            # during the overlapped attention of the other chunk).
            def s67_tiles(c):
                otc8 = st[c]["otc"]  # [128, KH, CHUNK] fp8
                mixN = st[c]["mixN"]
                ych = ws.tile([128, NT, G], f32, tag=f"hvy{c}", name="ych")
                mvall = sm.tile([128, NT, 2], f32, tag="mvall", bufs=2,
                                name="mvall")
                iva = sm.tile([128, NT], f32, tag="iva", bufs=2, name="iva")

                def stats_t(t):
                    ops_ = psum_big()
                    for n in range(2):
                        for g in range(4):
                            nc.tensor.matmul(
                                ops_[:, n * 512:(n + 1) * 512],
                                otc8[:, 2 * g:2 * g + 2, t * 128:(t + 1) * 128],
                                wo_sb[:, 2 * g:2 * g + 2, n * 512:(n + 1) * 512],
                                start=(g == 0), stop=(g == 3), perf_mode=DR)
                    res = mixN[:, t, :]
                    nc.vector.scalar_tensor_tensor(res, ops_[:], INV_WO, res,
                                                   op0=Alu.mult, op1=Alu.add)
                    st6 = sm.tile([128, 2, 6], f32, tag="st6b")
                    for half in range(2):
                        nc.vector.bn_stats(st6[:, half, :],
                                           mixN[:, t, half * 512:(half + 1) * 512])
                    nc.vector.bn_aggr(mvall[:, t, :], st6[:])

                def batch_iv(hh):
                    sq = sm.tile([128, 4], f32, tag="sqb", name="sqb")
                    nc.scalar.activation(sq[:], mvall[:, hh * 4:(hh + 1) * 4, 1],
                                         Act.Sqrt, bias=eps_sb[:])
                    nc.vector.reciprocal_approx_fast(iva[:, hh * 4:(hh + 1) * 4],
                                                     sq[:])

                def norm_t(t):
                    res = mixN[:, t, :]
                    nc.gpsimd.tensor_scalar(res, res, mvall[:, t, 0:1],
                                            iva[:, t:t + 1],
                                            op0=Alu.subtract, op1=Alu.mult)
                    zst = sm.tile([128, KH, 128], fp16, tag="zst", bufs=2)
                    nc.sync.dma_start_transpose(zst[:], res)
                    yps = psum_big()[:, :G]
                    for i in range(KH):
                        nc.tensor.matmul(yps[:], zst[:, i, :],
                                         gw_sb[:, i, :],
                                         start=(i == 0), stop=(i == KH - 1))
                    nc.vector.tensor_tensor(ych[:, t, :], yps[:], bw_sb[:],
                                            op=Alu.add)
                    if debug and c == 0 and t == NT - 1:
                        nc.sync.dma_start(dbg["dres"].ap(), mixN[:])

                def fin():
                    for hh in range(2):
                        nc.sync.dma_start(
                            y.ap()[c, hh * 512:(hh + 1) * 512, :].rearrange(
                                "(t p) g -> p t g", p=128),
                            ych[:, hh * 4:(hh + 1) * 4, :])

                out = []
                for hh in range(2):
                    for t in range(4 * hh, 4 * hh + 4):
                        out.append(lambda t=t: stats_t(t))
                    out.append(lambda hh=hh: batch_iv(hh))
                    for t in range(4 * hh, 4 * hh + 4):
                        out.append(lambda t=t: norm_t(t))
                return out + [fin]

            # ---------------- schedule ----------------
            from collections import deque

            STAGE_MARKS.clear()

            def mark(label):
                n = int(nc.get_next_instruction_name().split("-")[1])
                STAGE_MARKS.append((label, n))

            def mk_drain(q, keep=0):
                state = {"i": 0}

                def drain(n):
                    state["i"] += 1
                    if keep and state["i"] % keep == 0:
                        return
                    for _ in range(min(n, len(q))):
                        q.popleft()()
                return drain

            a, b = 0, 1
            mark("s01a")
            s01_load(a)
            s01_load(b)
            load_big_weights()
            s01_mm(a)
            mark("s2a")
            qa = deque(s2_tiles(a))
            while qa:
                qa.popleft()()
            mark("s01b")
            s01_mm(b)
            # A.S3 + A.S4 with B.S2 interleaved across the whole stream
            mark("s34a+s2b")
            qb = deque(s2_tiles(b))
            for th in s3_tiles(a) + s4_tiles(a):
                th()
                if qb:
                    qb.popleft()()
            while qb:
                qb.popleft()()
            # A.S5 with B.S3 + B.S4 as filler
            mark("s5a+s34b")
            qb = deque(s3_tiles(b) + s4_tiles(b))
            drain_b = mk_drain(qb)
            for h in range(NUM_HEADS):
                s5_head(a, h, drain_b)
            while qb:
                qb.popleft()()
            s5_fin(a)
            # B.S5 with A.S6+S7 as filler
            mark("s5b+s67a")
            qa = deque(s67_tiles(a))
            drain_a = mk_drain(qa)
            for h in range(NUM_HEADS):
                s5_head(b, h, drain_a)
            while qa:
                qa.popleft()()
            s5_fin(b)
            # B tail
            mark("s67b")
            for th in s67_tiles(b):
                th()
            mark("end")

    nc.compile()
    return nc


def _get_compiled():
    global _COMPILED
    if _COMPILED is None:
        _COMPILED = _build()
    return _COMPILED


def _prep_inputs(inputs):
    f32 = np.float32

    def a(name):
        return np.asarray(inputs[name], dtype=f32)

    x = a("x")
    mw = a("mother_wavelets")
    scales = a("scales")
    norm = np.sqrt(np.sum(mw ** 2, axis=2, keepdims=True))
    kern = (mw / np.maximum(norm, 1e-12)) * (1.0 / (1.0 + np.exp(-scales)))
    kern = kern[0, :, :, 0]                      # (W, H)
    kernT = np.ascontiguousarray(kern.T).astype(F16)

    w1a = np.concatenate([a("mix_w1"), a("mix_b1")[None, :]], axis=0).astype(F16)
    gln = np.ascontiguousarray(a("mix_ln_g").reshape(KM, 128).T).astype(f32)
    bln = np.ascontiguousarray(a("mix_ln_b").reshape(KM, 128).T).astype(f32)
    w2 = a("mix_w2").astype(F16)
    b2c = np.ascontiguousarray(a("mix_b2").reshape(KH, 128).T).astype(f32)
    gw = (a("out_ln_g")[:, None] * a("out_w")).astype(F16)
    bw_vec = a("out_ln_b") @ a("out_w") + a("out_b")
    bw = np.tile(bw_vec[None, :], (128, 1)).astype(f32)
    smallw = np.concatenate([gln, bln, b2c, bw], axis=1).astype(f32)

    def to8(w, s):
        ws = w * s
        am = np.abs(ws).max()
        assert am < 224.0, f"fp8 overflow: {am}"
        return ws.astype(F8)

    wq = to8(a("wq"), S_W8)
    wk = to8(a("wk"), S_W8)
    wv = to8(a("wv"), S_W8)
    wo = to8(a("wo"), S_W8)

    shared = {
        "kernt": kernT, "w1a": w1a, "smallw": smallw, "w2": w2,
        "wq8": wq, "wk8": wk, "wv8": wv, "wo8": wo, "gw": gw,
    }

    xc = x.reshape(N_CHUNKS, CHUNK, H)
    xt_all = np.ascontiguousarray(xc.transpose(0, 2, 1)).astype(F16)
    in_maps = []
    for core in range(N_CORES):
        m = dict(shared)
        m["xt"] = np.ascontiguousarray(xt_all[core * CPC:(core + 1) * CPC])
        in_maps.append(m)
    return in_maps


def kernel(**inputs) -> np.ndarray:
    from concourse.bass_utils import run_bass_kernel_spmd

    nc = _get_compiled()
    in_maps = _prep_inputs(inputs)
    res = run_bass_kernel_spmd(nc, in_maps, core_ids=list(range(N_CORES)))
    out = np.concatenate([r["y"] for r in res.results], axis=0)  # (16, CHUNK, G)
    return out.reshape(B, S, G).astype(np.float32)
